# revision 1
# baseline (speedup 1.0000x reference)
"""GAT layer (LayerNorm -> GATConv(heads=1) -> residual ReLU) on 8 trn2 NeuronCores.

Sharding: destination-node (graph/data) parallel. Each core owns a contiguous
range of N/8 nodes: it computes the node transform for its shard, the shards
are AllGathered so every core holds the full transformed-node table, and each
core then processes the edges whose destination falls in its shard.

Per destination block of 128 nodes, source-node records are fetched with
dma_gather (768 B rows: [xp+bias | 1 | a_src | pad]), per-edge a_dst with a
second dma_gather from a core-local 256 B-row table, attention weights
ee = exp(leakyrelu(a_src + a_dst)) are computed on DVE/ACT, and the
scatter-add is a one-hot matmul: lhsT[e, r] = (iota_r == dstlocal_e) * ee_e
accumulated into PSUM; the table's ones-column yields the softmax denominator
in the same matmuls. Attention/norm parameters are folded on the host into a
single [D,131] matrix + affine row and replicated to every core.
"""

import numpy as np

import concourse.bacc as bacc
import concourse.mybir as mybir
import concourse.tile as tile
from concourse.bass_utils import run_bass_kernel_spmd

F32 = mybir.dt.float32
I16 = mybir.dt.int16
AX = mybir.AxisListType
OP = mybir.AluOpType
AF = mybir.ActivationFunctionType

N = 50000
D = 128
E = 600000
NCORES = 8
SHARD = N // NCORES            # 6250
NBLK = (SHARD + 127) // 128    # 49 dst blocks per core
PAD_SHARD = NBLK * 128         # 6272
LAST_ROWS = SHARD - (NBLK - 1) * 128  # 106
FROW = 192                     # table row f32s (768 B, dma_gather granularity)
AROW = 64                      # a_dst table row f32s (256 B)
GCOL = 130                     # matmul rhs columns: [feat(128) | 1 | a_src]
COL_ONE = 128
COL_ASRC = 129
HALF = 32768                   # int16 index split point for the global table
NEG_SLOPE = 0.2
LN_EPS = 1e-5
GBLK = 2                       # dst blocks per gather group
DEBUG_MAX_GROUPS = None        # limit phase-B groups (bisection aid)
DEBUG_STAGE = 4                # 1=gathers 2=+ee 3=+matmul 4=full (bisection aid)
DEBUG_NO_AG = False            # replace AllGather with local copy (bisection aid)
DEBUG_NO_PHASE_A = False       # stub out phase-A compute (bisection aid)
DEBUG_GATHERS = "both"         # "feat" | "adst" | "both" (bisection aid)


def _build_program(tlo, thi):
    """One SPMD program; per-core behaviour differs only through its inputs.

    tlo/thi: per-block tile counts (of 128 edge slots) for the low/high
    halves of the source table, uniform across cores.
    """
    nc = bacc.Bacc("TRN2", num_devices=NCORES, debug=False)

    CB = sum(tlo) + sum(thi)   # total column-blocks (tiles) per core

    x_shard = nc.dram_tensor("x_shard", [PAD_SHARD, D], F32, kind="ExternalInput")
    wext = nc.dram_tensor("wext", [D, 131], F32, kind="ExternalInput")
    c2b = nc.dram_tensor("c2b", [128, 131], F32, kind="ExternalInput")
    ident = nc.dram_tensor("ident", [128, 128], F32, kind="ExternalInput")
    iota = nc.dram_tensor("iota", [128, 128], F32, kind="ExternalInput")
    feat_idx = nc.dram_tensor("feat_idx", [128, CB * 8], I16, kind="ExternalInput")
    adst_idx = nc.dram_tensor("adst_idx", [128, CB * 8], I16, kind="ExternalInput")
    dloc = nc.dram_tensor("dloc", [128, CB], F32, kind="ExternalInput")
    out_shard = nc.dram_tensor("out_shard", [SHARD, D], F32, kind="ExternalOutput")

    # group structure (static, identical on every core)
    groups = []
    cb0 = 0
    for g0 in range(0, NBLK, GBLK):
        blocks = list(range(g0, min(NBLK, g0 + GBLK)))
        nlo = sum(tlo[b] for b in blocks)
        nhi = sum(thi[b] for b in blocks)
        groups.append((blocks, cb0, nlo, nhi))
        cb0 += nlo + nhi
    assert cb0 == CB
    CBG_MAX = max(nlo + nhi for _, _, nlo, nhi in groups)

    with tile.TileContext(nc) as tc:
        with (
            tc.tile_pool(name="dram", bufs=1, space="DRAM") as dram,
            tc.tile_pool(name="consts", bufs=1) as cpool,
            tc.tile_pool(name="xres", bufs=1) as xpool,
        ):
            xp_shard = dram.tile([SHARD, FROW], F32)
            xp_full = dram.tile([N, FROW], F32, addr_space="Shared")
            adst_loc = dram.tile([PAD_SHARD, AROW], F32)

            ident_sb = cpool.tile([128, 128], F32)
            nc.sync.dma_start(ident_sb[:], ident[:, :])
            iota_sb = cpool.tile([128, 128], F32)
            nc.sync.dma_start(iota_sb[:], iota[:, :])
            wext_sb = cpool.tile([D, 131], F32)
            nc.sync.dma_start(wext_sb[:], wext[:, :])
            c2b_sb = cpool.tile([128, 131], F32)
            nc.sync.dma_start(c2b_sb[:], c2b[:, :])
            eps_sb = cpool.tile([128, 1], F32)
            nc.vector.memset(eps_sb[:], LN_EPS)
            fidx_sb = cpool.tile([128, CB * 8], I16)
            nc.sync.dma_start(fidx_sb[:], feat_idx[:, :])
            aidx_sb = cpool.tile([128, CB * 8], I16)
            nc.sync.dma_start(aidx_sb[:], adst_idx[:, :])
            dl_sb = cpool.tile([128, CB], F32)
            nc.sync.dma_start(dl_sb[:], dloc[:, :])

            x_tiles = []
            for i in range(NBLK):
                xt = xpool.tile([128, D], F32, tag=f"xres{i}")
                nc.sync.dma_start(xt[:], x_shard[i * 128 : (i + 1) * 128, :])
                x_tiles.append(xt)

            # ---------------- Phase A: node transform on own shard ---------
            if DEBUG_NO_PHASE_A:
                nc.sync.dma_start(xp_shard[:, 0:D], x_shard[0:SHARD, :])
                nc.sync.dma_start(
                    adst_loc[0:SHARD, 0:1], x_shard[0:SHARD, 0:1]
                )
            with (
                tc.tile_pool(name="a_small", bufs=8) as spool,
                tc.tile_pool(name="a_sq", bufs=2) as sqpool,
                tc.tile_pool(name="a_xnp", bufs=3) as xnppool,
                tc.tile_pool(name="a_xnpT", bufs=3) as xnptpool,
                tc.tile_pool(name="a_xpe", bufs=3) as xpepool,
                tc.tile_pool(name="a_ps_t", bufs=2, space="PSUM") as psa,
                tc.tile_pool(name="a_ps_m", bufs=2, space="PSUM") as psb,
            ):
                for i in range(NBLK if not DEBUG_NO_PHASE_A else 0):
                    xt = x_tiles[i]
                    rows = 128 if i < NBLK - 1 else LAST_ROWS
                    sumx = spool.tile([128, 1], F32, tag="sumx")
                    nc.vector.tensor_reduce(sumx[:], xt[:], AX.X, OP.add)
                    sqj = sqpool.tile([128, D], F32)
                    ssq = spool.tile([128, 1], F32, tag="ssq")
                    nc.scalar.activation(sqj[:], xt[:], AF.Square, accum_out=ssq[:])
                    mu = spool.tile([128, 1], F32, tag="mu")
                    nc.vector.tensor_scalar(mu[:], sumx[:], 1.0 / D, None, OP.mult)
                    m2 = spool.tile([128, 1], F32, tag="m2")
                    nc.vector.tensor_tensor(m2[:], mu[:], mu[:], OP.mult)
                    var = spool.tile([128, 1], F32, tag="var")
                    nc.vector.tensor_scalar(
                        var[:], ssq[:], 1.0 / D, m2[:, 0:1], OP.mult, OP.subtract
                    )
                    std = spool.tile([128, 1], F32, tag="std")
                    nc.scalar.activation(std[:], var[:], AF.Sqrt, bias=eps_sb[:, 0:1])
                    rstd = spool.tile([128, 1], F32, tag="rstd")
                    nc.vector.reciprocal(rstd[:], std[:])
                    xnp = xnppool.tile([128, D], F32)
                    nc.vector.tensor_scalar(
                        xnp[:], xt[:], mu[:, 0:1], rstd[:, 0:1], OP.subtract, OP.mult
                    )
                    pt = psa.tile([128, 128], F32, space="PSUM")
                    nc.tensor.transpose(pt[:], xnp[:], ident_sb[:])
                    xnpT = xnptpool.tile([128, 128], F32)
                    nc.scalar.copy(xnpT[:], pt[:])
                    pm = psb.tile([128, 131], F32, space="PSUM")
                    nc.tensor.matmul(
                        pm[:], lhsT=xnpT[:], rhs=wext_sb[:], start=True, stop=True
                    )
                    xpe = xpepool.tile([128, 131], F32)
                    nc.vector.tensor_tensor(xpe[:], pm[:], c2b_sb[:], OP.add)
                    nc.sync.dma_start(
                        xp_shard[i * 128 : i * 128 + rows, 0:130], xpe[:rows, 0:130]
                    )
                    nc.sync.dma_start(
                        adst_loc[i * 128 : i * 128 + rows, 0:1], xpe[:rows, 130:131]
                    )

            if DEBUG_NO_AG:
                nc.sync.dma_start(xp_full[0:SHARD, :], xp_shard[:, :])
            else:
                nc.gpsimd.collective_compute(
                    "AllGather",
                    OP.bypass,
                    replica_groups=[list(range(NCORES))],
                    ins=[xp_shard[:, :]],
                    outs=[xp_full[:, :]],
                )

            # ---------------- Phase B: edge aggregation --------------------
            with (
                tc.tile_pool(name="b_g", bufs=2) as gpool,
                tc.tile_pool(name="b_a", bufs=2) as apool,
                tc.tile_pool(name="b_sw", bufs=4) as swpool,
                tc.tile_pool(name="b_e", bufs=3) as epool,
                tc.tile_pool(name="b_ep", bufs=3) as eppool,
                tc.tile_pool(name="b_ps", bufs=4, space="PSUM") as psc,
            ):
                use_groups = groups if DEBUG_MAX_GROUPS is None else groups[:DEBUG_MAX_GROUPS]
                for blocks, cb0, nlo, nhi in use_groups:
                    cbg = nlo + nhi
                    gf = gpool.tile([128, CBG_MAX, FROW], F32, tag="gf")
                    if DEBUG_GATHERS == "adst":
                        nc.vector.memset(gf.rearrange("p a b -> p (a b)")[:], 0.0)
                    if nlo and DEBUG_GATHERS in ("feat", "both"):
                        nc.gpsimd.dma_gather(
                            out_ap=gf[:, 0:nlo, :],
                            in_ap=xp_full[0:HALF, :],
                            idxs_ap=fidx_sb[:, cb0 * 8 : (cb0 + nlo) * 8],
                            num_idxs=nlo * 128,
                            num_idxs_reg=nlo * 128,
                            elem_size=FROW,
                            single_packet=False,
                        )
                    if nhi and DEBUG_GATHERS in ("feat", "both"):
                        nc.gpsimd.dma_gather(
                            out_ap=gf[:, nlo:cbg, :],
                            in_ap=xp_full[HALF:N, :],
                            idxs_ap=fidx_sb[:, (cb0 + nlo) * 8 : (cb0 + cbg) * 8],
                            num_idxs=nhi * 128,
                            num_idxs_reg=nhi * 128,
                            elem_size=FROW,
                            single_packet=False,
                        )
                    ga = apool.tile([128, CBG_MAX, AROW], F32, tag="ga")
                    if DEBUG_GATHERS == "feat":
                        nc.vector.memset(ga.rearrange("p a b -> p (a b)")[:], 1.0)
                    if DEBUG_GATHERS in ("adst", "both"):
                      nc.gpsimd.dma_gather(
                        out_ap=ga[:, 0:cbg, :],
                        in_ap=adst_loc[:, :],
                        idxs_ap=aidx_sb[:, cb0 * 8 : (cb0 + cbg) * 8],
                        num_idxs=cbg * 128,
                        num_idxs_reg=cbg * 128,
                        elem_size=AROW,
                        single_packet=False,
                    )
                    if DEBUG_STAGE < 2:
                        for b in blocks:
                            rows = 128 if b < NBLK - 1 else LAST_ROWS
                            nc.sync.dma_start(
                                out_shard[b * 128 : b * 128 + rows, :],
                                gf[:rows, (b - blocks[0]), 0:D],
                            )
                        continue
                    # ee = exp(leakyrelu(a_src + a_dst)) for the whole group
                    e1 = epool.tile([128, CBG_MAX], F32, tag="e1")
                    nc.vector.tensor_tensor(
                        e1[:, 0:cbg], gf[:, 0:cbg, COL_ASRC], ga[:, 0:cbg, 0], OP.add
                    )
                    e2 = epool.tile([128, CBG_MAX], F32, tag="e2")
                    nc.vector.tensor_scalar(
                        e2[:, 0:cbg], e1[:, 0:cbg], NEG_SLOPE, None, OP.mult
                    )
                    e3 = epool.tile([128, CBG_MAX], F32, tag="e3")
                    nc.vector.tensor_tensor(
                        e3[:, 0:cbg], e2[:, 0:cbg], e1[:, 0:cbg], OP.max
                    )
                    ee = epool.tile([128, CBG_MAX], F32, tag="ee")
                    nc.scalar.activation(ee[:, 0:cbg], e3[:, 0:cbg], AF.Exp)
                    if DEBUG_STAGE < 3:
                        for b in blocks:
                            rows = 128 if b < NBLK - 1 else LAST_ROWS
                            tmp = eppool.tile([128, D], F32, tag="outt")
                            nc.vector.tensor_scalar(
                                tmp[:], iota_sb[:],
                                ee[:, (b - blocks[0]) : (b - blocks[0]) + 1],
                                None, OP.mult,
                            )
                            nc.sync.dma_start(
                                out_shard[b * 128 : b * 128 + rows, :], tmp[:rows, :]
                            )
                        continue

                    # per-block one-hot scatter matmuls
                    lo_off = 0
                    hi_off = nlo
                    for b in blocks:
                        rows = 128 if b < NBLK - 1 else LAST_ROWS
                        cbs = list(range(lo_off, lo_off + tlo[b])) + list(
                            range(hi_off, hi_off + thi[b])
                        )
                        lo_off += tlo[b]
                        hi_off += thi[b]
                        ps = psc.tile([128, GCOL], F32, space="PSUM")
                        for j, cb in enumerate(cbs):
                            sw = swpool.tile([128, 128], F32)
                            nc.vector.tensor_scalar(
                                sw[:],
                                iota_sb[:],
                                dl_sb[:, cb0 + cb : cb0 + cb + 1],
                                ee[:, cb : cb + 1],
                                OP.is_equal,
                                OP.mult,
                            )
                            nc.tensor.matmul(
                                ps[:, :],
                                lhsT=sw[:],
                                rhs=gf[:, cb, 0:GCOL],
                                start=(j == 0),
                                stop=(j == len(cbs) - 1),
                            )
                        if DEBUG_STAGE < 4:
                            tmp = eppool.tile([128, D], F32, tag="outt")
                            nc.vector.tensor_copy(tmp[:], ps[:, 0:D])
                            nc.sync.dma_start(
                                out_shard[b * 128 : b * 128 + rows, :], tmp[:rows, :]
                            )
                            continue
                        recip = epool.tile([128, 1], F32, tag="recip")
                        nc.vector.reciprocal(recip[:], ps[:, COL_ONE : COL_ONE + 1])
                        scaled = eppool.tile([128, D], F32, tag="scaled")
                        nc.scalar.activation(
                            scaled[:], ps[:, 0:D], AF.Copy, scale=recip[:, 0:1]
                        )
                        resid = eppool.tile([128, D], F32, tag="resid")
                        nc.vector.tensor_tensor(
                            resid[:], scaled[:], x_tiles[b][:], OP.add
                        )
                        outt = eppool.tile([128, D], F32, tag="outt")
                        nc.scalar.activation(outt[:], resid[:], AF.Relu)
                        nc.sync.dma_start(
                            out_shard[b * 128 : b * 128 + rows, :], outt[:rows, :]
                        )

    nc.compile()
    return nc


def _wrap_idx(idx):
    """int16 index list -> dma_gather SBUF layout [128, len/16]:
    index i lives at partitions {16g + i%16: g in 0..7}, column i//16."""
    L = len(idx)
    assert L % 16 == 0
    w = idx.reshape(L // 16, 16).T.astype(np.int16)      # [16, L/16]
    return np.tile(w, (8, 1))                            # [128, L/16]


def _host_prep(x, edge_index, ln_gamma, ln_beta, W, att_src, att_dst, bias):
    """Fold parameters and bucket edges by destination block. Numpy only."""
    Wt = W.T.astype(np.float64)
    G = ln_gamma.astype(np.float64)[:, None] * Wt          # [D, D]
    crow = ln_beta.astype(np.float64) @ Wt                 # [D]
    v_src = G @ att_src.astype(np.float64)
    v_dst = G @ att_dst.astype(np.float64)
    c_src = float(crow @ att_src.astype(np.float64))
    c_dst = float(crow @ att_dst.astype(np.float64))

    wext = np.zeros((D, 131), np.float32)
    wext[:, 0:D] = G.astype(np.float32)
    wext[:, COL_ASRC] = v_src.astype(np.float32)
    wext[:, 130] = v_dst.astype(np.float32)
    c2 = np.zeros((131,), np.float32)
    c2[0:D] = (crow + bias.astype(np.float64)).astype(np.float32)
    c2[COL_ONE] = 1.0
    c2[COL_ASRC] = c_src
    c2[130] = c_dst
    c2b = np.broadcast_to(c2, (128, 131)).copy()

    ident = np.eye(128, dtype=np.float32)
    iota = np.broadcast_to(np.arange(128, dtype=np.float32), (128, 128)).copy()

    # edges + self loops, sorted by (core, block, src-half)
    src = np.concatenate([edge_index[0], np.arange(N, dtype=np.int64)]).astype(np.int64)
    dst = np.concatenate([edge_index[1], np.arange(N, dtype=np.int64)]).astype(np.int64)
    core = dst // SHARD
    local = dst - core * SHARD
    blk = local // 128
    half = (src >= HALF).astype(np.int64)
    key = ((core * NBLK + blk) * 2 + half)
    order = np.argsort(key, kind="stable")
    src, dst, key = src[order], dst[order], key[order]
    counts = np.bincount(key, minlength=NCORES * NBLK * 2).reshape(NCORES, NBLK, 2)
    tiles = -(-counts // 128)                              # ceil
    tlo = tuple(int(t) for t in tiles[:, :, 0].max(axis=0))
    thi = tuple(int(t) for t in tiles[:, :, 1].max(axis=0))
    CB = sum(tlo) + sum(thi)

    # per-core slot tables in global column-block (cb) order
    feat_idx = np.zeros((NCORES, CB * 128), np.int16)
    adst_idx = np.zeros((NCORES, CB * 128), np.int16)
    dloc = np.full((NCORES, 128, CB), 128.0, np.float32)

    starts = np.zeros(NCORES * NBLK * 2 + 1, np.int64)
    starts[1:] = np.cumsum(counts.reshape(-1))

    # cb offset of each (block, half) segment, same for every core
    seg_off = {}
    cb0 = 0
    for g0 in range(0, NBLK, GBLK):
        blocks = list(range(g0, min(NBLK, g0 + GBLK)))
        off = cb0
        for b in blocks:
            seg_off[(b, 0)] = off
            off += tlo[b]
        for b in blocks:
            seg_off[(b, 1)] = off
            off += thi[b]
        cb0 = off
    assert cb0 == CB

    for c in range(NCORES):
        for b in range(NBLK):
            for hf in range(2):
                gi = (c * NBLK + b) * 2 + hf
                s, e = starts[gi], starts[gi + 1]
                n = int(e - s)
                if n == 0:
                    continue
                off = seg_off[(b, hf)]
                k = np.arange(n) + off * 128
                fi = (src[s:e] - hf * HALF).astype(np.int16)
                feat_idx[c, k] = fi
                ai = (dst[s:e] - c * SHARD).astype(np.int16)
                adst_idx[c, k] = ai
                p = k % 128
                t = k // 128
                dloc[c, p, t] = (dst[s:e] - (c * SHARD + b * 128)).astype(np.float32)

    in_maps = []
    for c in range(NCORES):
        xs = np.zeros((PAD_SHARD, D), np.float32)
        xs[0:SHARD] = x[c * SHARD : (c + 1) * SHARD]
        in_maps.append(
            {
                "x_shard": xs,
                "wext": wext,
                "c2b": c2b,
                "ident": ident,
                "iota": iota,
                "feat_idx": _wrap_idx(feat_idx[c]),
                "adst_idx": _wrap_idx(adst_idx[c]),
                "dloc": np.ascontiguousarray(dloc[c]),
            }
        )
    return tlo, thi, in_maps


_PROGRAM_CACHE = {}


def kernel(x, edge_index, edge_attr, h, batch, ln_gamma, ln_beta, W, att_src,
           att_dst, bias):
    x = np.asarray(x, dtype=np.float32)
    edge_index = np.asarray(edge_index)
    h = np.asarray(h)
    ln_gamma = np.asarray(ln_gamma, dtype=np.float32)
    ln_beta = np.asarray(ln_beta, dtype=np.float32)
    W = np.asarray(W, dtype=np.float32)
    att_src = np.asarray(att_src, dtype=np.float32)
    att_dst = np.asarray(att_dst, dtype=np.float32)
    bias = np.asarray(bias, dtype=np.float32)

    tlo, thi, in_maps = _host_prep(
        x, edge_index, ln_gamma, ln_beta, W, att_src, att_dst, bias
    )
    key = (tlo, thi)
    if key not in _PROGRAM_CACHE:
        _PROGRAM_CACHE[key] = _build_program(tlo, thi)
    nc = _PROGRAM_CACHE[key]

    res = run_bass_kernel_spmd(nc, in_maps, core_ids=list(range(NCORES)))
    out = np.concatenate([res.results[c]["out_shard"] for c in range(NCORES)], axis=0)
    return out, h



# revision 2
# speedup vs baseline: 1.8700x; 1.8700x over previous
"""GAT layer (LayerNorm -> GATConv(heads=1) -> residual ReLU) on 8 trn2 NeuronCores.

Sharding: destination-node parallel. Each core owns N/8 contiguous nodes,
computes the node transform for its shard, AllGathers the transformed table,
then processes the edges whose destination lands in its shard.

Key design points (v2, rebuilt after profiling the 768B-row baseline):
- The node table is fp16, 256 B/row (the dma_gather minimum): a host-side
  orthonormal rotation Q puts att_src along coordinate 127, so the gathered
  row IS [rotated feats | a_src] with zero extra columns; the rotation is
  undone after the softmax-weighted scatter by one 128x128 matmul per dst
  block (Q is orthogonal, applied to the accumulated sums).
- No per-edge a_dst gather: a_dst per edge = ohT_cb^T @ adst_block via a
  1-column matmul per 128-edge column block, with one-hot tables streamed
  from the host as fp8 (exact 0/1).
- No DVE one-hot builds: the scatter matmul uses lhsT = host fp8 one-hot,
  rhs = gathered rows * ee (folded on ACT/DVE), with a constant ones column
  in the rhs producing the softmax denominator in the same matmul.
- Feature gathers round-robin over 4 SWDGE queues to overlap DMA drains.
"""

import numpy as np
import ml_dtypes

import concourse.bacc as bacc
import concourse.mybir as mybir
import concourse.tile as tile
from concourse.bass_utils import run_bass_kernel_spmd

F32 = mybir.dt.float32
F16 = mybir.dt.float16
F8 = mybir.dt.float8e4
I16 = mybir.dt.int16
AX = mybir.AxisListType
OP = mybir.AluOpType
AF = mybir.ActivationFunctionType

N = 50000
D = 128
E = 600000
NCORES = 8
SHARD = N // NCORES            # 6250
NBLK = (SHARD + 127) // 128    # 49 dst blocks per core
PAD_SHARD = NBLK * 128         # 6272
LAST_ROWS = SHARD - (NBLK - 1) * 128  # 106
HALF = 32768                   # int16 index split point for the global table
NEG_SLOPE = 0.2
LN_EPS = 1e-5
GBLK = 4                       # dst blocks per gather group
NQ = 4                         # SWDGE queues for gathers


def _build_program(tlo, thi, na1):
    """One SPMD program; per-core behaviour differs only through its inputs."""
    nc = bacc.Bacc("TRN2", num_devices=NCORES, debug=False, num_swdge_queues=NQ)

    CB = sum(tlo) + sum(thi)   # total column-blocks (tiles) per core

    x_shard = nc.dram_tensor("x_shard", [PAD_SHARD, D], F32, kind="ExternalInput")
    wextq = nc.dram_tensor("wextq", [D, 129], F32, kind="ExternalInput")
    c2q = nc.dram_tensor("c2q", [128, 129], F32, kind="ExternalInput")
    ident = nc.dram_tensor("ident", [128, 128], F32, kind="ExternalInput")
    qmat = nc.dram_tensor("qmat", [128, 128], F16, kind="ExternalInput")
    feat_idx = nc.dram_tensor("feat_idx", [128, CB * 8], I16, kind="ExternalInput")
    oh_d = nc.dram_tensor("oh_d", [128, CB * 128], F8, kind="ExternalInput")
    ohT_d = nc.dram_tensor("ohT_d", [128, CB * 128], F8, kind="ExternalInput")
    out_shard = nc.dram_tensor("out_shard", [SHARD, D], F32, kind="ExternalOutput")

    # group structure (static, identical on every core)
    groups = []
    cb0 = 0
    for g0 in range(0, NBLK, GBLK):
        blocks = list(range(g0, min(NBLK, g0 + GBLK)))
        nlo = sum(tlo[b] for b in blocks)
        nhi = sum(thi[b] for b in blocks)
        groups.append((blocks, cb0, nlo, nhi))
        cb0 += nlo + nhi
    assert cb0 == CB
    CBG_MAX = max(nlo + nhi for _, _, nlo, nhi in groups)

    # cb -> owning block (within its group), same order the host uses
    cb_block = [0] * CB
    for blocks, cb0g, nlo, nhi in groups:
        off = cb0g
        for b in blocks:
            for _ in range(tlo[b]):
                cb_block[off] = b
                off += 1
        for b in blocks:
            for _ in range(thi[b]):
                cb_block[off] = b
                off += 1

    with tile.TileContext(nc) as tc:
        with (
            tc.tile_pool(name="dram", bufs=1, space="DRAM") as dram,
            tc.tile_pool(name="consts", bufs=1) as cpool,
            tc.tile_pool(name="xres", bufs=1) as xpool,
            tc.tile_pool(name="gfp", bufs=1) as gfppool,
        ):
            xb_shard = dram.tile([SHARD, D], F16)
            xb_full = dram.tile([N, D], F16, addr_space="Shared")

            ident_sb = cpool.tile([128, 128], F32)
            nc.sync.dma_start(ident_sb[:], ident[:, :])
            q_sb = cpool.tile([128, 128], F16)
            nc.sync.dma_start(q_sb[:], qmat[:, :])
            wext_sb = cpool.tile([D, 129], F32)
            nc.sync.dma_start(wext_sb[:], wextq[:, :])
            c2_sb = cpool.tile([128, 129], F32)
            nc.sync.dma_start(c2_sb[:], c2q[:, :])
            eps_sb = cpool.tile([128, 1], F32)
            nc.vector.memset(eps_sb[:], LN_EPS)
            fidx_sb = cpool.tile([128, CB * 8], I16)
            nc.sync.dma_start(fidx_sb[:], feat_idx[:, :])
            adst_sb = cpool.tile([128, NBLK], F16)

            x_tiles = []
            for i in range(NBLK):
                xt = xpool.tile([128, D], F32, tag=f"xres{i}")
                nc.sync.dma_start(xt[:], x_shard[i * 128 : (i + 1) * 128, :])
                x_tiles.append(xt)

            # gfp double buffers: [rot feats(128) | 1 | 1], ones cols persist
            gfpA = gfppool.tile([128, CBG_MAX, 130], F16, tag="gfpA")
            gfpB = gfppool.tile([128, CBG_MAX, 130], F16, tag="gfpB")
            nc.vector.memset(gfpA.rearrange("p a b -> p (a b)")[:], 1.0)
            nc.vector.memset(gfpB.rearrange("p a b -> p (a b)")[:], 1.0)
            gfps = [gfpA, gfpB]

            # ---------------- Phase A: node transform on own shard ---------
            with (
                tc.tile_pool(name="a_small", bufs=8) as spool,
                tc.tile_pool(name="a_sq", bufs=2) as sqpool,
                tc.tile_pool(name="a_xnp", bufs=3) as xnppool,
                tc.tile_pool(name="a_xnpT", bufs=3) as xnptpool,
                tc.tile_pool(name="a_xpe", bufs=3) as xpepool,
                tc.tile_pool(name="a_tb", bufs=3) as tbpool,
                tc.tile_pool(name="a_ps_t", bufs=2, space="PSUM") as psa,
                tc.tile_pool(name="a_ps_m", bufs=2, space="PSUM") as psb,
            ):
                for i in range(NBLK):
                    xt = x_tiles[i]
                    rows = 128 if i < NBLK - 1 else LAST_ROWS
                    sumx = spool.tile([128, 1], F32, tag="sumx")
                    nc.vector.tensor_reduce(sumx[:], xt[:], AX.X, OP.add)
                    sqj = sqpool.tile([128, D], F32)
                    ssq = spool.tile([128, 1], F32, tag="ssq")
                    nc.scalar.activation(sqj[:], xt[:], AF.Square, accum_out=ssq[:])
                    mu = spool.tile([128, 1], F32, tag="mu")
                    nc.vector.tensor_scalar(mu[:], sumx[:], 1.0 / D, None, OP.mult)
                    m2 = spool.tile([128, 1], F32, tag="m2")
                    nc.vector.tensor_tensor(m2[:], mu[:], mu[:], OP.mult)
                    var = spool.tile([128, 1], F32, tag="var")
                    nc.vector.tensor_scalar(
                        var[:], ssq[:], 1.0 / D, m2[:, 0:1], OP.mult, OP.subtract
                    )
                    std = spool.tile([128, 1], F32, tag="std")
                    nc.scalar.activation(std[:], var[:], AF.Sqrt, bias=eps_sb[:, 0:1])
                    rstd = spool.tile([128, 1], F32, tag="rstd")
                    nc.vector.reciprocal(rstd[:], std[:])
                    xnp = xnppool.tile([128, D], F32)
                    nc.vector.tensor_scalar(
                        xnp[:], xt[:], mu[:, 0:1], rstd[:, 0:1], OP.subtract, OP.mult
                    )
                    pt = psa.tile([128, 128], F32, space="PSUM")
                    nc.tensor.transpose(pt[:], xnp[:], ident_sb[:])
                    xnpT = xnptpool.tile([128, 128], F32)
                    nc.scalar.copy(xnpT[:], pt[:])
                    pm = psb.tile([128, 129], F32, space="PSUM")
                    nc.tensor.matmul(
                        pm[:], lhsT=xnpT[:], rhs=wext_sb[:], start=True, stop=True
                    )
                    xpe = xpepool.tile([128, 129], F32)
                    nc.vector.tensor_tensor(xpe[:], pm[:], c2_sb[:], OP.add)
                    tb = tbpool.tile([128, D], F16, tag="tb")
                    nc.scalar.copy(tb[:], xpe[:, 0:128])
                    nc.sync.dma_start(
                        xb_shard[i * 128 : i * 128 + rows, :], tb[:rows, :]
                    )
                    nc.scalar.copy(adst_sb[:, i : i + 1], xpe[:, 128:129])

            nc.gpsimd.collective_compute(
                "AllGather",
                OP.bypass,
                replica_groups=[list(range(NCORES))],
                ins=[xb_shard[:, :]],
                outs=[xb_full[:, :]],
            )

            # ---------------- Phase B: edge aggregation --------------------
            with (
                tc.tile_pool(name="b_g", bufs=2) as gpool,
                tc.tile_pool(name="b_oh", bufs=2) as ohpool,
                tc.tile_pool(name="b_ohT", bufs=2) as ohtpool,
                tc.tile_pool(name="b_e", bufs=2) as epool,
                tc.tile_pool(name="b_gfe", bufs=4) as gfepool,
                tc.tile_pool(name="b_blk", bufs=4) as blkpool,
                tc.tile_pool(name="b_ps_a", bufs=2, space="PSUM") as ps_adst,
                tc.tile_pool(name="b_ps_s", bufs=2, space="PSUM") as ps_sc,
                tc.tile_pool(name="b_ps_t", bufs=2, space="PSUM") as ps_tp,
                tc.tile_pool(name="b_ps_o", bufs=2, space="PSUM") as ps_out,
            ):
                qctr = 0
                for gi, (blocks, cb0, nlo, nhi) in enumerate(groups):
                    cbg = nlo + nhi
                    gfp = gfps[gi % 2]
                    gf = gpool.tile([128, CBG_MAX, 128], F16, tag="gf")
                    if nlo:
                        nc.gpsimd.dma_gather(
                            out_ap=gf[:, 0:nlo, :],
                            in_ap=xb_full[0:HALF, :],
                            idxs_ap=fidx_sb[:, cb0 * 8 : (cb0 + nlo) * 8],
                            num_idxs=nlo * 128,
                            num_idxs_reg=nlo * 128,
                            elem_size=128,
                            single_packet=False,
                            queue_num=qctr % NQ,
                        )
                        qctr += 1
                    if nhi:
                        nc.gpsimd.dma_gather(
                            out_ap=gf[:, nlo:cbg, :],
                            in_ap=xb_full[HALF:N, :],
                            idxs_ap=fidx_sb[:, (cb0 + nlo) * 8 : (cb0 + cbg) * 8],
                            num_idxs=nhi * 128,
                            num_idxs_reg=nhi * 128,
                            elem_size=128,
                            single_packet=False,
                            queue_num=qctr % NQ,
                        )
                        qctr += 1
                    oh = ohpool.tile([128, CBG_MAX, 128], F8, tag="oh")
                    nc.sync.dma_start(
                        oh.rearrange("p a b -> p (a b)")[:, 0 : cbg * 128],
                        oh_d[:, cb0 * 128 : (cb0 + cbg) * 128],
                    )
                    ohT = ohtpool.tile([128, CBG_MAX, 128], F8, tag="ohT")
                    nc.sync.dma_start(
                        ohT.rearrange("p a b -> p (a b)")[:, 0 : cbg * 128],
                        ohT_d[:, cb0 * 128 : (cb0 + cbg) * 128],
                    )

                    # per-edge a_dst via transposed one-hot x per-block vector
                    pa = ps_adst.tile([128, CBG_MAX], F32, space="PSUM")
                    for j in range(cbg):
                        nc.tensor.matmul(
                            pa[:, j : j + 1],
                            lhsT=ohT[:, j, :],
                            rhs=adst_sb[:, cb_block[cb0 + j] : cb_block[cb0 + j] + 1],
                            start=True,
                            stop=True,
                            skip_group_check=True,
                        )

                    # ee = exp(leakyrelu(|att_src|*t127 + a_dst'))
                    e1 = epool.tile([128, CBG_MAX], F32, tag="e1")
                    nc.vector.scalar_tensor_tensor(
                        e1[:, 0:cbg],
                        in0=gf[:, 0:cbg, 127],
                        scalar=float(na1),
                        in1=pa[:, 0:cbg],
                        op0=OP.mult,
                        op1=OP.add,
                    )
                    e3 = epool.tile([128, CBG_MAX], F32, tag="e3")
                    nc.vector.tensor_scalar(
                        e3[:, 0:cbg], e1[:, 0:cbg], NEG_SLOPE, None, OP.mult
                    )
                    nc.vector.tensor_tensor(
                        e3[:, 0:cbg], e3[:, 0:cbg], e1[:, 0:cbg], OP.max
                    )
                    ee = epool.tile([128, CBG_MAX], F32, tag="ee")
                    nc.scalar.activation(ee[:, 0:cbg], e3[:, 0:cbg], AF.Exp)

                    # copy gathered rows into the ones-padded rhs buffer
                    nc.vector.tensor_copy(
                        gfp[:, 0:cbg, 0:128], gf[:, 0:cbg, :]
                    )

                    # scatter matmuls per block
                    lo_off = 0
                    hi_off = nlo
                    for b in blocks:
                        rows = 128 if b < NBLK - 1 else LAST_ROWS
                        cbs = list(range(lo_off, lo_off + tlo[b])) + list(
                            range(hi_off, hi_off + thi[b])
                        )
                        lo_off += tlo[b]
                        hi_off += thi[b]
                        ps = ps_sc.tile([128, 129], F32, space="PSUM")
                        for j, cb in enumerate(cbs):
                            gfe = gfepool.tile([128, 129], F16, tag="gfe")
                            if cb % 3 == 2:
                                nc.vector.tensor_scalar(
                                    gfe[:],
                                    gfp[:, cb, 0:129],
                                    ee[:, cb : cb + 1],
                                    None,
                                    OP.mult,
                                )
                            else:
                                nc.scalar.activation(
                                    gfe[:], gfp[:, cb, 0:129], AF.Copy,
                                    scale=ee[:, cb : cb + 1],
                                )
                            nc.tensor.matmul(
                                ps[:, :],
                                lhsT=oh[:, cb, :],
                                rhs=gfe[:],
                                start=(j == 0),
                                stop=(j == len(cbs) - 1),
                            )
                        recip = epool.tile([128, 1], F32, tag="recip")
                        nc.vector.reciprocal(recip[:], ps[:, 128:129])
                        scaled = blkpool.tile([128, D], F32, tag="scaled")
                        nc.scalar.activation(
                            scaled[:], ps[:, 0:D], AF.Copy, scale=recip[:, 0:1]
                        )
                        ptp = ps_tp.tile([128, 128], F32, space="PSUM")
                        nc.tensor.transpose(ptp[:], scaled[:], ident_sb[:])
                        scaledT = blkpool.tile([128, D], F16, tag="scaledT")
                        nc.scalar.copy(scaledT[:], ptp[:])
                        po = ps_out.tile([128, 128], F32, space="PSUM")
                        nc.tensor.matmul(
                            po[:], lhsT=scaledT[:], rhs=q_sb[:], start=True, stop=True
                        )
                        resid = blkpool.tile([128, D], F32, tag="resid")
                        nc.vector.tensor_tensor(
                            resid[:], po[:], x_tiles[b][:], OP.add
                        )
                        outt = blkpool.tile([128, D], F32, tag="outt")
                        nc.scalar.activation(outt[:], resid[:], AF.Relu)
                        nc.sync.dma_start(
                            out_shard[b * 128 : b * 128 + rows, :], outt[:rows, :]
                        )

    nc.compile()
    return nc


def _wrap_idx(idx):
    """int16 index list -> dma_gather SBUF layout [128, len/16]:
    index i lives at partitions {16g + i%16: g in 0..7}, column i//16."""
    L = len(idx)
    assert L % 16 == 0
    w = idx.reshape(L // 16, 16).T.astype(np.int16)      # [16, L/16]
    return np.tile(w, (8, 1))                            # [128, L/16]


def _host_prep(x, edge_index, ln_gamma, ln_beta, W, att_src, att_dst, bias):
    """Fold parameters, build rotation Q, bucket edges. Numpy only."""
    Wt = W.T.astype(np.float64)
    G = ln_gamma.astype(np.float64)[:, None] * Wt          # [D, D]
    crow = ln_beta.astype(np.float64) @ Wt                 # [D]
    a1 = att_src.astype(np.float64)
    a2 = att_dst.astype(np.float64)
    na1 = float(np.linalg.norm(a1))
    v_dst = G @ a2
    c_dst = float(crow @ a2)
    kc = float(bias.astype(np.float64) @ a1)

    # orthonormal Q with row 127 = att_src direction (row 126: att_dst comp,
    # kept only so Q is deterministic/well-conditioned)
    q127 = a1 / na1
    u = a2 - (a2 @ q127) * q127
    nu = np.linalg.norm(u)
    if nu > 1e-12:
        q126 = u / nu
        P = np.eye(D) - np.outer(q127, q127) - np.outer(q126, q126)
        Uq, _, _ = np.linalg.svd(P)
        Q = np.vstack([Uq[:, :126].T, q126[None, :], q127[None, :]])
    else:
        P = np.eye(D) - np.outer(q127, q127)
        Uq, _, _ = np.linalg.svd(P)
        Q = np.vstack([Uq[:, :127].T, q127[None, :]])

    c2feat = crow + bias.astype(np.float64)
    wextq = np.zeros((D, 129), np.float32)
    wextq[:, 0:128] = (G @ Q.T).astype(np.float32)
    wextq[:, 128] = v_dst.astype(np.float32)
    c2 = np.zeros((129,), np.float32)
    c2[0:128] = (c2feat @ Q.T).astype(np.float32)
    c2[128] = c_dst - kc
    c2b = np.broadcast_to(c2, (128, 129)).copy()

    ident = np.eye(128, dtype=np.float32)
    qmat = Q.astype(np.float16)

    # edges + self loops, sorted by (core, block, src-half)
    src = np.concatenate([edge_index[0], np.arange(N, dtype=np.int64)]).astype(np.int64)
    dst = np.concatenate([edge_index[1], np.arange(N, dtype=np.int64)]).astype(np.int64)
    core = dst // SHARD
    local = dst - core * SHARD
    blk = local // 128
    half = (src >= HALF).astype(np.int64)
    key = ((core * NBLK + blk) * 2 + half)
    order = np.argsort(key, kind="stable")
    src, dst, key = src[order], dst[order], key[order]
    counts = np.bincount(key, minlength=NCORES * NBLK * 2).reshape(NCORES, NBLK, 2)
    tiles = -(-counts // 128)                              # ceil
    tlo = tuple(int(t) for t in tiles[:, :, 0].max(axis=0))
    thi = tuple(int(t) for t in tiles[:, :, 1].max(axis=0))
    CB = sum(tlo) + sum(thi)

    feat_idx = np.zeros((NCORES, CB * 128), np.int16)
    oh = np.zeros((NCORES, 128, CB, 128), np.uint8)
    ohT = np.zeros((NCORES, 128, CB, 128), np.uint8)

    starts = np.zeros(NCORES * NBLK * 2 + 1, np.int64)
    starts[1:] = np.cumsum(counts.reshape(-1))

    # cb offset of each (block, half) segment, same for every core
    seg_off = {}
    cb0 = 0
    for g0 in range(0, NBLK, GBLK):
        blocks = list(range(g0, min(NBLK, g0 + GBLK)))
        off = cb0
        for b in blocks:
            seg_off[(b, 0)] = off
            off += tlo[b]
        for b in blocks:
            seg_off[(b, 1)] = off
            off += thi[b]
        cb0 = off
    assert cb0 == CB

    for c in range(NCORES):
        for b in range(NBLK):
            for hf in range(2):
                gi = (c * NBLK + b) * 2 + hf
                s, e = starts[gi], starts[gi + 1]
                n = int(e - s)
                if n == 0:
                    continue
                off = seg_off[(b, hf)]
                k = np.arange(n) + off * 128
                fi = (src[s:e] - hf * HALF).astype(np.int16)
                feat_idx[c, k] = fi
                p = k % 128
                t = k // 128
                r = (dst[s:e] - (c * SHARD + b * 128)).astype(np.int64)
                oh[c, p, t, r] = 1
                ohT[c, r, t, p] = 1

    oh8 = oh.astype(ml_dtypes.float8_e4m3fn).reshape(NCORES, 128, CB * 128)
    ohT8 = ohT.astype(ml_dtypes.float8_e4m3fn).reshape(NCORES, 128, CB * 128)

    in_maps = []
    for c in range(NCORES):
        xs = np.zeros((PAD_SHARD, D), np.float32)
        xs[0:SHARD] = x[c * SHARD : (c + 1) * SHARD]
        in_maps.append(
            {
                "x_shard": xs,
                "wextq": wextq,
                "c2q": c2b,
                "ident": ident,
                "qmat": qmat,
                "feat_idx": _wrap_idx(feat_idx[c]),
                "oh_d": np.ascontiguousarray(oh8[c]),
                "ohT_d": np.ascontiguousarray(ohT8[c]),
            }
        )
    return tlo, thi, na1, in_maps


_PROGRAM_CACHE = {}


def kernel(x, edge_index, edge_attr, h, batch, ln_gamma, ln_beta, W, att_src,
           att_dst, bias):
    x = np.asarray(x, dtype=np.float32)
    edge_index = np.asarray(edge_index)
    h = np.asarray(h)
    ln_gamma = np.asarray(ln_gamma, dtype=np.float32)
    ln_beta = np.asarray(ln_beta, dtype=np.float32)
    W = np.asarray(W, dtype=np.float32)
    att_src = np.asarray(att_src, dtype=np.float32)
    att_dst = np.asarray(att_dst, dtype=np.float32)
    bias = np.asarray(bias, dtype=np.float32)

    tlo, thi, na1, in_maps = _host_prep(
        x, edge_index, ln_gamma, ln_beta, W, att_src, att_dst, bias
    )
    key = (tlo, thi, round(na1, 6))
    if key not in _PROGRAM_CACHE:
        _PROGRAM_CACHE[key] = _build_program(tlo, thi, na1)
    nc = _PROGRAM_CACHE[key]

    res = run_bass_kernel_spmd(nc, in_maps, core_ids=list(range(NCORES)))
    out = np.concatenate([res.results[c]["out_shard"] for c in range(NCORES)], axis=0)
    return out, h


# revision 7
# speedup vs baseline: 2.4836x; 1.3282x over previous
"""GAT layer (LayerNorm -> GATConv(heads=1) -> residual ReLU) on 8 trn2 NeuronCores.

Sharding: destination-node parallel. Each core owns N/8 contiguous nodes,
computes the node transform for its shard, AllGathers the transformed table,
then processes the edges whose destination lands in its shard.

Key design points (v2, rebuilt after profiling the 768B-row baseline):
- The node table is fp16, 256 B/row (the dma_gather minimum): a host-side
  orthonormal rotation Q puts att_src along coordinate 127, so the gathered
  row IS [rotated feats | a_src] with zero extra columns; the rotation is
  undone after the softmax-weighted scatter by one 128x128 matmul per dst
  block (Q is orthogonal, applied to the accumulated sums).
- No per-edge a_dst gather: a_dst per edge = ohT_cb^T @ adst_block via a
  1-column matmul per 128-edge column block, with one-hot tables streamed
  from the host as fp8 (exact 0/1).
- No DVE one-hot builds: the scatter matmul uses lhsT = host fp8 one-hot,
  rhs = gathered rows * ee (folded on ACT/DVE), with a constant ones column
  in the rhs producing the softmax denominator in the same matmul.
- Feature gathers round-robin over 4 SWDGE queues to overlap DMA drains.
"""

import numpy as np
import ml_dtypes

import concourse.bacc as bacc
import concourse.mybir as mybir
import concourse.tile as tile
from concourse.bass_utils import run_bass_kernel_spmd

F32 = mybir.dt.float32
F16 = mybir.dt.float16
F8 = mybir.dt.float8e4
I16 = mybir.dt.int16
AX = mybir.AxisListType
OP = mybir.AluOpType
AF = mybir.ActivationFunctionType

N = 50000
D = 128
E = 600000
NCORES = 8
SHARD = N // NCORES            # 6250
NBLK = (SHARD + 127) // 128    # 49 dst blocks per core
PAD_SHARD = NBLK * 128         # 6272
LAST_ROWS = SHARD - (NBLK - 1) * 128  # 106
HALF = 32768                   # int16 index split point for the global table
NEG_SLOPE = 0.2
LN_EPS = 1e-5
GBLK = 4                       # dst blocks per gather group
NQ = 4                         # SWDGE queues for gathers


def _build_program(tlo, thi, na1):
    """One SPMD program; per-core behaviour differs only through its inputs."""
    nc = bacc.Bacc("TRN2", num_devices=NCORES, debug=False, num_swdge_queues=NQ)

    CB = sum(tlo) + sum(thi)   # total column-blocks (tiles) per core

    x_shard = nc.dram_tensor("x_shard", [PAD_SHARD, D], F32, kind="ExternalInput")
    wextq = nc.dram_tensor("wextq", [D, 129], F32, kind="ExternalInput")
    c2q = nc.dram_tensor("c2q", [128, 129], F32, kind="ExternalInput")
    ident = nc.dram_tensor("ident", [128, 128], F32, kind="ExternalInput")
    qmat = nc.dram_tensor("qmat", [128, 128], F16, kind="ExternalInput")
    feat_idx = nc.dram_tensor("feat_idx", [128, CB * 8], I16, kind="ExternalInput")
    oh_d = nc.dram_tensor("oh_d", [128, CB * 128], F8, kind="ExternalInput")
    ohT_d = nc.dram_tensor("ohT_d", [128, CB * 128], F8, kind="ExternalInput")
    out_shard = nc.dram_tensor("out_shard", [SHARD, D], F32, kind="ExternalOutput")

    # group structure (static, identical on every core)
    groups = []
    cb0 = 0
    for g0 in range(0, NBLK, GBLK):
        blocks = list(range(g0, min(NBLK, g0 + GBLK)))
        nlo = sum(tlo[b] for b in blocks)
        nhi = sum(thi[b] for b in blocks)
        groups.append((blocks, cb0, nlo, nhi))
        cb0 += nlo + nhi
    assert cb0 == CB
    CBG_MAX = max(nlo + nhi for _, _, nlo, nhi in groups)

    # cb -> owning block (within its group), same order the host uses
    cb_block = [0] * CB
    for blocks, cb0g, nlo, nhi in groups:
        off = cb0g
        for b in blocks:
            for _ in range(tlo[b]):
                cb_block[off] = b
                off += 1
        for b in blocks:
            for _ in range(thi[b]):
                cb_block[off] = b
                off += 1

    with tile.TileContext(nc) as tc:
        with (
            tc.tile_pool(name="dram", bufs=1, space="DRAM") as dram,
            tc.tile_pool(name="consts", bufs=1) as cpool,
            tc.tile_pool(name="xres", bufs=1) as xpool,
        ):
            xb_shard = dram.tile([SHARD, D], F16)
            xb_full = dram.tile([N, D], F16, addr_space="Shared")

            ident_sb = cpool.tile([128, 128], F32)
            nc.sync.dma_start(ident_sb[:], ident[:, :])
            q_sb = cpool.tile([128, 128], F16)
            nc.sync.dma_start(q_sb[:], qmat[:, :])
            wext_sb = cpool.tile([D, 129], F32)
            nc.sync.dma_start(wext_sb[:], wextq[:, :])
            c2_sb = cpool.tile([128, 129], F32)
            nc.sync.dma_start(c2_sb[:], c2q[:, :])
            eps_sb = cpool.tile([128, 1], F32)
            nc.vector.memset(eps_sb[:], LN_EPS)
            fidx_sb = cpool.tile([128, CB * 8], I16)
            nc.sync.dma_start(fidx_sb[:], feat_idx[:, :])
            adst_sb = cpool.tile([128, NBLK], F16)

            x_tiles = []
            for i in range(NBLK):
                xt = xpool.tile([128, D], F32, tag=f"xres{i}")
                nc.sync.dma_start(xt[:], x_shard[i * 128 : (i + 1) * 128, :])
                x_tiles.append(xt)

            # ---------------- Phase A: node transform on own shard ---------
            with (
                tc.tile_pool(name="a_small", bufs=8) as spool,
                tc.tile_pool(name="a_sq", bufs=2) as sqpool,
                tc.tile_pool(name="a_xnp", bufs=3) as xnppool,
                tc.tile_pool(name="a_xnpT", bufs=3) as xnptpool,
                tc.tile_pool(name="a_xpe", bufs=3) as xpepool,
                tc.tile_pool(name="a_tb", bufs=3) as tbpool,
                tc.tile_pool(name="a_ps_t", bufs=2, space="PSUM") as psa,
                tc.tile_pool(name="a_ps_m", bufs=2, space="PSUM") as psb,
            ):
                for i in range(NBLK):
                    xt = x_tiles[i]
                    rows = 128 if i < NBLK - 1 else LAST_ROWS
                    sumx = spool.tile([128, 1], F32, tag="sumx")
                    nc.vector.tensor_reduce(sumx[:], xt[:], AX.X, OP.add)
                    sqj = sqpool.tile([128, D], F32)
                    ssq = spool.tile([128, 1], F32, tag="ssq")
                    nc.scalar.activation(sqj[:], xt[:], AF.Square, accum_out=ssq[:])
                    mu = spool.tile([128, 1], F32, tag="mu")
                    nc.vector.tensor_scalar(mu[:], sumx[:], 1.0 / D, None, OP.mult)
                    m2 = spool.tile([128, 1], F32, tag="m2")
                    nc.vector.tensor_tensor(m2[:], mu[:], mu[:], OP.mult)
                    var = spool.tile([128, 1], F32, tag="var")
                    nc.vector.tensor_scalar(
                        var[:], ssq[:], 1.0 / D, m2[:, 0:1], OP.mult, OP.subtract
                    )
                    std = spool.tile([128, 1], F32, tag="std")
                    nc.scalar.activation(std[:], var[:], AF.Sqrt, bias=eps_sb[:, 0:1])
                    rstd = spool.tile([128, 1], F32, tag="rstd")
                    nc.vector.reciprocal(rstd[:], std[:])
                    xnp = xnppool.tile([128, D], F32)
                    nc.vector.tensor_scalar(
                        xnp[:], xt[:], mu[:, 0:1], rstd[:, 0:1], OP.subtract, OP.mult
                    )
                    pt = psa.tile([128, 128], F32, space="PSUM")
                    nc.tensor.transpose(pt[:], xnp[:], ident_sb[:])
                    xnpT = xnptpool.tile([128, 128], F32)
                    nc.scalar.copy(xnpT[:], pt[:])
                    pm = psb.tile([128, 129], F32, space="PSUM")
                    nc.tensor.matmul(
                        pm[:], lhsT=xnpT[:], rhs=wext_sb[:], start=True, stop=True
                    )
                    xpe = xpepool.tile([128, 129], F32)
                    nc.vector.tensor_tensor(xpe[:], pm[:], c2_sb[:], OP.add)
                    tb = tbpool.tile([128, D], F16, tag="tb")
                    nc.scalar.copy(tb[:], xpe[:, 0:128])
                    nc.sync.dma_start(
                        xb_shard[i * 128 : i * 128 + rows, :], tb[:rows, :]
                    )
                    nc.scalar.copy(adst_sb[:, i : i + 1], xpe[:, 128:129])

            nc.gpsimd.collective_compute(
                "AllGather",
                OP.bypass,
                replica_groups=[list(range(NCORES))],
                ins=[xb_shard[:, :]],
                outs=[xb_full[:, :]],
            )

            # ---------------- Phase B: edge aggregation --------------------
            with (
                tc.tile_pool(name="b_g", bufs=2) as gpool,
                tc.tile_pool(name="b_oh", bufs=2) as ohpool,
                tc.tile_pool(name="b_ohT", bufs=2) as ohtpool,
                tc.tile_pool(name="b_e", bufs=2) as epool,
                tc.tile_pool(name="b_gfe", bufs=2) as gfepool,
                tc.tile_pool(name="b_blk", bufs=4) as blkpool,
                tc.tile_pool(name="b_ps_a", bufs=2, space="PSUM") as ps_adst,
                tc.tile_pool(name="b_ps_s", bufs=2, space="PSUM") as ps_sc,
                tc.tile_pool(name="b_ps_t", bufs=2, space="PSUM") as ps_tp,
                tc.tile_pool(name="b_ps_o", bufs=2, space="PSUM") as ps_out,
            ):
                qctr = 0
                for gi, (blocks, cb0, nlo, nhi) in enumerate(groups):
                    cbg = nlo + nhi
                    gf = gpool.tile([128, CBG_MAX, 128], F16, tag="gf")
                    # split each half-table gather in two on separate SWDGE
                    # queues so their DMA drains overlap
                    segs = []
                    if nlo:
                        h1 = (nlo + 1) // 2
                        segs += [(0, h1, 0), (h1, nlo, 0)] if nlo > 1 else [(0, nlo, 0)]
                    if nhi:
                        h2 = (nhi + 1) // 2
                        segs += (
                            [(nlo, nlo + h2, 1), (nlo + h2, cbg, 1)]
                            if nhi > 1
                            else [(nlo, cbg, 1)]
                        )
                    for s0, s1, hf in segs:
                        nc.gpsimd.dma_gather(
                            out_ap=gf[:, s0:s1, :],
                            in_ap=xb_full[0:HALF, :] if hf == 0 else xb_full[HALF:N, :],
                            idxs_ap=fidx_sb[:, (cb0 + s0) * 8 : (cb0 + s1) * 8],
                            num_idxs=(s1 - s0) * 128,
                            num_idxs_reg=(s1 - s0) * 128,
                            elem_size=128,
                            single_packet=False,
                            queue_num=qctr % NQ,
                        )
                        qctr += 1
                    oh = ohpool.tile([128, CBG_MAX, 128], F8, tag="oh")
                    nc.sync.dma_start(
                        oh.rearrange("p a b -> p (a b)")[:, 0 : cbg * 128],
                        oh_d[:, cb0 * 128 : (cb0 + cbg) * 128],
                    )
                    ohT = ohtpool.tile([128, CBG_MAX, 128], F8, tag="ohT")
                    nc.sync.dma_start(
                        ohT.rearrange("p a b -> p (a b)")[:, 0 : cbg * 128],
                        ohT_d[:, cb0 * 128 : (cb0 + cbg) * 128],
                    )

                    # per-edge a_dst via transposed one-hot x per-block vector
                    pa = ps_adst.tile([128, CBG_MAX], F32, space="PSUM")
                    for j in range(cbg):
                        nc.tensor.matmul(
                            pa[:, j : j + 1],
                            lhsT=ohT[:, j, :],
                            rhs=adst_sb[:, cb_block[cb0 + j] : cb_block[cb0 + j] + 1],
                            start=True,
                            stop=True,
                            skip_group_check=True,
                        )

                    # ee = exp(leakyrelu(|att_src|*t127 + a_dst'))
                    e1 = epool.tile([128, CBG_MAX], F32, tag="e1")
                    nc.vector.scalar_tensor_tensor(
                        e1[:, 0:cbg],
                        in0=gf[:, 0:cbg, 127],
                        scalar=float(na1),
                        in1=pa[:, 0:cbg],
                        op0=OP.mult,
                        op1=OP.add,
                    )
                    e3 = epool.tile([128, CBG_MAX], F32, tag="e3")
                    nc.vector.tensor_scalar(
                        e3[:, 0:cbg], e1[:, 0:cbg], NEG_SLOPE, None, OP.mult
                    )
                    nc.vector.tensor_tensor(
                        e3[:, 0:cbg], e3[:, 0:cbg], e1[:, 0:cbg], OP.max
                    )
                    ee = epool.tile([128, CBG_MAX], F32, tag="ee")
                    nc.scalar.activation(ee[:, 0:cbg], e3[:, 0:cbg], AF.Exp)

                    # rhs for the scatter matmuls: [t*ee (128) | ee | pad],
                    # built in one batched multiply + one strided column copy
                    gfe = gfepool.tile([128, CBG_MAX, 130], F16, tag="gfe")
                    nc.vector.tensor_tensor(
                        gfe[:, 0:cbg, 0:128],
                        gf[:, 0:cbg, :],
                        ee[:, 0:cbg].to_broadcast([128, cbg, 128]),
                        OP.mult,
                    )
                    nc.vector.tensor_copy(gfe[:, 0:cbg, 128], ee[:, 0:cbg])

                    # scatter matmuls per block
                    lo_off = 0
                    hi_off = nlo
                    for b in blocks:
                        rows = 128 if b < NBLK - 1 else LAST_ROWS
                        cbs = list(range(lo_off, lo_off + tlo[b])) + list(
                            range(hi_off, hi_off + thi[b])
                        )
                        lo_off += tlo[b]
                        hi_off += thi[b]
                        ps = ps_sc.tile([128, 129], F32, space="PSUM")
                        for j, cb in enumerate(cbs):
                            nc.tensor.matmul(
                                ps[:, :],
                                lhsT=oh[:, cb, :],
                                rhs=gfe[:, cb, 0:129],
                                start=(j == 0),
                                stop=(j == len(cbs) - 1),
                            )
                        recip = epool.tile([128, 1], F32, tag="recip")
                        nc.vector.reciprocal(recip[:], ps[:, 128:129])
                        scaled = blkpool.tile([128, D], F32, tag="scaled")
                        nc.scalar.activation(
                            scaled[:], ps[:, 0:D], AF.Copy, scale=recip[:, 0:1]
                        )
                        ptp = ps_tp.tile([128, 128], F32, space="PSUM")
                        nc.tensor.transpose(ptp[:], scaled[:], ident_sb[:])
                        scaledT = blkpool.tile([128, D], F16, tag="scaledT")
                        nc.scalar.copy(scaledT[:], ptp[:])
                        po = ps_out.tile([128, 128], F32, space="PSUM")
                        nc.tensor.matmul(
                            po[:], lhsT=scaledT[:], rhs=q_sb[:], start=True, stop=True
                        )
                        resid = blkpool.tile([128, D], F32, tag="resid")
                        nc.vector.tensor_tensor(
                            resid[:], po[:], x_tiles[b][:], OP.add
                        )
                        outt = blkpool.tile([128, D], F32, tag="outt")
                        nc.scalar.activation(outt[:], resid[:], AF.Relu)
                        nc.sync.dma_start(
                            out_shard[b * 128 : b * 128 + rows, :], outt[:rows, :]
                        )

    nc.compile()
    return nc


def _wrap_idx(idx):
    """int16 index list -> dma_gather SBUF layout [128, len/16]:
    index i lives at partitions {16g + i%16: g in 0..7}, column i//16."""
    L = len(idx)
    assert L % 16 == 0
    w = idx.reshape(L // 16, 16).T.astype(np.int16)      # [16, L/16]
    return np.tile(w, (8, 1))                            # [128, L/16]


def _host_prep(x, edge_index, ln_gamma, ln_beta, W, att_src, att_dst, bias):
    """Fold parameters, build rotation Q, bucket edges. Numpy only."""
    Wt = W.T.astype(np.float64)
    G = ln_gamma.astype(np.float64)[:, None] * Wt          # [D, D]
    crow = ln_beta.astype(np.float64) @ Wt                 # [D]
    a1 = att_src.astype(np.float64)
    a2 = att_dst.astype(np.float64)
    na1 = float(np.linalg.norm(a1))
    v_dst = G @ a2
    c_dst = float(crow @ a2)
    kc = float(bias.astype(np.float64) @ a1)

    # orthonormal Q with row 127 = att_src direction (row 126: att_dst comp,
    # kept only so Q is deterministic/well-conditioned)
    q127 = a1 / na1
    u = a2 - (a2 @ q127) * q127
    nu = np.linalg.norm(u)
    if nu > 1e-12:
        q126 = u / nu
        P = np.eye(D) - np.outer(q127, q127) - np.outer(q126, q126)
        Uq, _, _ = np.linalg.svd(P)
        Q = np.vstack([Uq[:, :126].T, q126[None, :], q127[None, :]])
    else:
        P = np.eye(D) - np.outer(q127, q127)
        Uq, _, _ = np.linalg.svd(P)
        Q = np.vstack([Uq[:, :127].T, q127[None, :]])

    c2feat = crow + bias.astype(np.float64)
    wextq = np.zeros((D, 129), np.float32)
    wextq[:, 0:128] = (G @ Q.T).astype(np.float32)
    wextq[:, 128] = v_dst.astype(np.float32)
    c2 = np.zeros((129,), np.float32)
    c2[0:128] = (c2feat @ Q.T).astype(np.float32)
    c2[128] = c_dst - kc
    c2b = np.broadcast_to(c2, (128, 129)).copy()

    ident = np.eye(128, dtype=np.float32)
    qmat = Q.astype(np.float16)

    # edges + self loops, sorted by (core, block, src-half)
    src = np.concatenate([edge_index[0], np.arange(N, dtype=np.int64)]).astype(np.int64)
    dst = np.concatenate([edge_index[1], np.arange(N, dtype=np.int64)]).astype(np.int64)
    core = dst // SHARD
    local = dst - core * SHARD
    blk = local // 128
    half = (src >= HALF).astype(np.int64)
    key = ((core * NBLK + blk) * 2 + half)
    order = np.argsort(key, kind="stable")
    src, dst, key = src[order], dst[order], key[order]
    counts = np.bincount(key, minlength=NCORES * NBLK * 2).reshape(NCORES, NBLK, 2)
    tiles = -(-counts // 128)                              # ceil
    tlo = tuple(int(t) for t in tiles[:, :, 0].max(axis=0))
    thi = tuple(int(t) for t in tiles[:, :, 1].max(axis=0))
    CB = sum(tlo) + sum(thi)

    feat_idx = np.zeros((NCORES, CB * 128), np.int16)
    oh = np.zeros((NCORES, 128, CB, 128), np.uint8)
    ohT = np.zeros((NCORES, 128, CB, 128), np.uint8)

    starts = np.zeros(NCORES * NBLK * 2 + 1, np.int64)
    starts[1:] = np.cumsum(counts.reshape(-1))

    # cb offset of each (block, half) segment, same for every core
    seg_off = {}
    cb0 = 0
    for g0 in range(0, NBLK, GBLK):
        blocks = list(range(g0, min(NBLK, g0 + GBLK)))
        off = cb0
        for b in blocks:
            seg_off[(b, 0)] = off
            off += tlo[b]
        for b in blocks:
            seg_off[(b, 1)] = off
            off += thi[b]
        cb0 = off
    assert cb0 == CB

    for c in range(NCORES):
        for b in range(NBLK):
            for hf in range(2):
                gi = (c * NBLK + b) * 2 + hf
                s, e = starts[gi], starts[gi + 1]
                n = int(e - s)
                if n == 0:
                    continue
                off = seg_off[(b, hf)]
                k = np.arange(n) + off * 128
                fi = (src[s:e] - hf * HALF).astype(np.int16)
                feat_idx[c, k] = fi
                p = k % 128
                t = k // 128
                r = (dst[s:e] - (c * SHARD + b * 128)).astype(np.int64)
                oh[c, p, t, r] = 1
                ohT[c, r, t, p] = 1

    oh8 = oh.astype(ml_dtypes.float8_e4m3fn).reshape(NCORES, 128, CB * 128)
    ohT8 = ohT.astype(ml_dtypes.float8_e4m3fn).reshape(NCORES, 128, CB * 128)

    in_maps = []
    for c in range(NCORES):
        xs = np.zeros((PAD_SHARD, D), np.float32)
        xs[0:SHARD] = x[c * SHARD : (c + 1) * SHARD]
        in_maps.append(
            {
                "x_shard": xs,
                "wextq": wextq,
                "c2q": c2b,
                "ident": ident,
                "qmat": qmat,
                "feat_idx": _wrap_idx(feat_idx[c]),
                "oh_d": np.ascontiguousarray(oh8[c]),
                "ohT_d": np.ascontiguousarray(ohT8[c]),
            }
        )
    return tlo, thi, na1, in_maps


_PROGRAM_CACHE = {}


def kernel(x, edge_index, edge_attr, h, batch, ln_gamma, ln_beta, W, att_src,
           att_dst, bias):
    x = np.asarray(x, dtype=np.float32)
    edge_index = np.asarray(edge_index)
    h = np.asarray(h)
    ln_gamma = np.asarray(ln_gamma, dtype=np.float32)
    ln_beta = np.asarray(ln_beta, dtype=np.float32)
    W = np.asarray(W, dtype=np.float32)
    att_src = np.asarray(att_src, dtype=np.float32)
    att_dst = np.asarray(att_dst, dtype=np.float32)
    bias = np.asarray(bias, dtype=np.float32)

    tlo, thi, na1, in_maps = _host_prep(
        x, edge_index, ln_gamma, ln_beta, W, att_src, att_dst, bias
    )
    key = (tlo, thi, round(na1, 6))
    if key not in _PROGRAM_CACHE:
        _PROGRAM_CACHE[key] = _build_program(tlo, thi, na1)
    nc = _PROGRAM_CACHE[key]

    res = run_bass_kernel_spmd(nc, in_maps, core_ids=list(range(NCORES)))
    out = np.concatenate([res.results[c]["out_shard"] for c in range(NCORES)], axis=0)
    return out, h


# revision 12
# speedup vs baseline: 2.5038x; 1.0081x over previous
"""GAT layer (LayerNorm -> GATConv(heads=1) -> residual ReLU) on 8 trn2 NeuronCores.

Sharding: destination-node parallel. Each core owns N/8 contiguous nodes,
computes the node transform for its shard, AllGathers the transformed table,
then processes the edges whose destination lands in its shard.

Key design points (v2, rebuilt after profiling the 768B-row baseline):
- The node table is fp16, 256 B/row (the dma_gather minimum): a host-side
  orthonormal rotation Q puts att_src along coordinate 127, so the gathered
  row IS [rotated feats | a_src] with zero extra columns; the rotation is
  undone after the softmax-weighted scatter by one 128x128 matmul per dst
  block (Q is orthogonal, applied to the accumulated sums).
- No per-edge a_dst gather: a_dst per edge = ohT_cb^T @ adst_block via a
  1-column matmul per 128-edge column block, with one-hot tables streamed
  from the host as fp8 (exact 0/1).
- No DVE one-hot builds: the scatter matmul uses lhsT = host fp8 one-hot,
  rhs = gathered rows * ee (folded on ACT/DVE), with a constant ones column
  in the rhs producing the softmax denominator in the same matmul.
- Feature gathers round-robin over 4 SWDGE queues to overlap DMA drains.
"""

import numpy as np
import ml_dtypes

import concourse.bacc as bacc
import concourse.mybir as mybir
import concourse.tile as tile
from concourse.bass_utils import run_bass_kernel_spmd

F32 = mybir.dt.float32
F16 = mybir.dt.float16
F8 = mybir.dt.float8e4
I16 = mybir.dt.int16
AX = mybir.AxisListType
OP = mybir.AluOpType
AF = mybir.ActivationFunctionType

N = 50000
D = 128
E = 600000
NCORES = 8
SHARD = N // NCORES            # 6250
NBLK = (SHARD + 127) // 128    # 49 dst blocks per core
PAD_SHARD = NBLK * 128         # 6272
LAST_ROWS = SHARD - (NBLK - 1) * 128  # 106
HALF = 32768                   # int16 index split point for the global table
NEG_SLOPE = 0.2
LN_EPS = 1e-5
GBLK = 4                       # dst blocks per gather group
NQ = 4                         # SWDGE queues for gathers


def _build_program(tlo, thi, na1):
    """One SPMD program; per-core behaviour differs only through its inputs."""
    nc = bacc.Bacc("TRN2", num_devices=NCORES, debug=False, num_swdge_queues=NQ)

    CB = sum(tlo) + sum(thi)   # total column-blocks (tiles) per core

    x_shard = nc.dram_tensor("x_shard", [PAD_SHARD, D], F32, kind="ExternalInput")
    wextq = nc.dram_tensor("wextq", [D, 129], F16, kind="ExternalInput")
    c2q = nc.dram_tensor("c2q", [128, 129], F32, kind="ExternalInput")
    ident = nc.dram_tensor("ident", [128, 128], F32, kind="ExternalInput")
    qmat = nc.dram_tensor("qmat", [128, 128], F16, kind="ExternalInput")
    feat_idx = nc.dram_tensor("feat_idx", [128, CB * 8], I16, kind="ExternalInput")
    oh_d = nc.dram_tensor("oh_d", [128, CB * 128], F8, kind="ExternalInput")
    ohT_d = nc.dram_tensor("ohT_d", [128, CB * 128], F8, kind="ExternalInput")
    out_shard = nc.dram_tensor("out_shard", [SHARD, D], F32, kind="ExternalOutput")

    # group structure (static, identical on every core)
    groups = []
    cb0 = 0
    for g0 in range(0, NBLK, GBLK):
        blocks = list(range(g0, min(NBLK, g0 + GBLK)))
        nlo = sum(tlo[b] for b in blocks)
        nhi = sum(thi[b] for b in blocks)
        groups.append((blocks, cb0, nlo, nhi))
        cb0 += nlo + nhi
    assert cb0 == CB
    CBG_MAX = max(nlo + nhi for _, _, nlo, nhi in groups)

    # cb -> owning block (within its group), same order the host uses
    cb_block = [0] * CB
    for blocks, cb0g, nlo, nhi in groups:
        off = cb0g
        for b in blocks:
            for _ in range(tlo[b]):
                cb_block[off] = b
                off += 1
        for b in blocks:
            for _ in range(thi[b]):
                cb_block[off] = b
                off += 1

    with tile.TileContext(nc) as tc:
        with (
            tc.tile_pool(name="dram", bufs=1, space="DRAM") as dram,
            tc.tile_pool(name="consts", bufs=1) as cpool,
            tc.tile_pool(name="xres", bufs=1) as xpool,
        ):
            xb_shard = dram.tile([SHARD, D], F16)
            xb_full = dram.tile([N, D], F16, addr_space="Shared")

            ident_sb = cpool.tile([128, 128], F32)
            nc.sync.dma_start(ident_sb[:], ident[:, :])
            q_sb = cpool.tile([128, 128], F16)
            nc.sync.dma_start(q_sb[:], qmat[:, :])
            wext_sb = cpool.tile([D, 129], F16)
            nc.sync.dma_start(wext_sb[:], wextq[:, :])
            identh_sb = cpool.tile([128, 128], F16)
            nc.scalar.copy(identh_sb[:], ident_sb[:])
            c2_sb = cpool.tile([128, 129], F32)
            nc.sync.dma_start(c2_sb[:], c2q[:, :])
            eps_sb = cpool.tile([128, 1], F32)
            nc.vector.memset(eps_sb[:], LN_EPS)
            fidx_sb = cpool.tile([128, CB * 8], I16)
            nc.sync.dma_start(fidx_sb[:], feat_idx[:, :])
            adst_sb = cpool.tile([128, NBLK], F16)

            x_tiles = []
            for i in range(NBLK):
                xt = xpool.tile([128, D], F32, tag=f"xres{i}")
                nc.sync.dma_start(xt[:], x_shard[i * 128 : (i + 1) * 128, :])
                x_tiles.append(xt)

            # ---------------- Phase A: node transform on own shard ---------
            with (
                tc.tile_pool(name="a_small", bufs=8) as spool,
                tc.tile_pool(name="a_sq", bufs=2) as sqpool,
                tc.tile_pool(name="a_xnp", bufs=3) as xnppool,
                tc.tile_pool(name="a_xnpT", bufs=3) as xnptpool,
                tc.tile_pool(name="a_xpe", bufs=3) as xpepool,
                tc.tile_pool(name="a_tb", bufs=3) as tbpool,
                tc.tile_pool(name="a_ps_t", bufs=2, space="PSUM") as psa,
                tc.tile_pool(name="a_ps_m", bufs=2, space="PSUM") as psb,
            ):
                for i in range(NBLK):
                    xt = x_tiles[i]
                    rows = 128 if i < NBLK - 1 else LAST_ROWS
                    sumx = spool.tile([128, 1], F32, tag="sumx")
                    nc.vector.tensor_reduce(sumx[:], xt[:], AX.X, OP.add)
                    sqj = sqpool.tile([128, D], F32)
                    ssq = spool.tile([128, 1], F32, tag="ssq")
                    nc.scalar.activation(sqj[:], xt[:], AF.Square, accum_out=ssq[:])
                    mu = spool.tile([128, 1], F32, tag="mu")
                    nc.vector.tensor_scalar(mu[:], sumx[:], 1.0 / D, None, OP.mult)
                    m2 = spool.tile([128, 1], F32, tag="m2")
                    nc.vector.tensor_tensor(m2[:], mu[:], mu[:], OP.mult)
                    var = spool.tile([128, 1], F32, tag="var")
                    nc.vector.tensor_scalar(
                        var[:], ssq[:], 1.0 / D, m2[:, 0:1], OP.mult, OP.subtract
                    )
                    std = spool.tile([128, 1], F32, tag="std")
                    nc.scalar.activation(std[:], var[:], AF.Sqrt, bias=eps_sb[:, 0:1])
                    rstd = spool.tile([128, 1], F32, tag="rstd")
                    nc.vector.reciprocal(rstd[:], std[:])
                    xnp = xnppool.tile([128, D], F16)
                    nc.vector.tensor_scalar(
                        xnp[:], xt[:], mu[:, 0:1], rstd[:, 0:1], OP.subtract, OP.mult
                    )
                    pt = psa.tile([128, 128], F16, space="PSUM")
                    nc.tensor.transpose(pt[:], xnp[:], identh_sb[:])
                    xnpT = xnptpool.tile([128, 128], F16)
                    nc.scalar.copy(xnpT[:], pt[:])
                    pm = psb.tile([128, 129], F32, space="PSUM")
                    nc.tensor.matmul(
                        pm[:], lhsT=xnpT[:], rhs=wext_sb[:], start=True, stop=True
                    )
                    xpe = xpepool.tile([128, 129], F32)
                    nc.vector.tensor_tensor(xpe[:], pm[:], c2_sb[:], OP.add)
                    tb = tbpool.tile([128, D], F16, tag="tb")
                    nc.scalar.copy(tb[:], xpe[:, 0:128])
                    nc.sync.dma_start(
                        xb_shard[i * 128 : i * 128 + rows, :], tb[:rows, :]
                    )
                    nc.scalar.copy(adst_sb[:, i : i + 1], xpe[:, 128:129])

            nc.gpsimd.collective_compute(
                "AllGather",
                OP.bypass,
                replica_groups=[list(range(NCORES))],
                ins=[xb_shard[:, :]],
                outs=[xb_full[:, :]],
            )

            # ---------------- Phase B: edge aggregation --------------------
            with (
                tc.tile_pool(name="b_g", bufs=3) as gpool,
                tc.tile_pool(name="b_oh", bufs=2) as ohpool,
                tc.tile_pool(name="b_ohT", bufs=2) as ohtpool,
                tc.tile_pool(name="b_e", bufs=2) as epool,
                tc.tile_pool(name="b_gfe", bufs=2) as gfepool,
                tc.tile_pool(name="b_blk", bufs=4) as blkpool,
                tc.tile_pool(name="b_ps_a", bufs=2, space="PSUM") as ps_adst,
                tc.tile_pool(name="b_ps_s", bufs=2, space="PSUM") as ps_sc,
                tc.tile_pool(name="b_ps_t", bufs=2, space="PSUM") as ps_tp,
                tc.tile_pool(name="b_ps_o", bufs=2, space="PSUM") as ps_out,
            ):
                qctr = 0
                for gi, (blocks, cb0, nlo, nhi) in enumerate(groups):
                    cbg = nlo + nhi
                    gf = gpool.tile([128, CBG_MAX, 128], F16, tag="gf")
                    # split each half-table gather in two on separate SWDGE
                    # queues so their DMA drains overlap
                    segs = []
                    if nlo:
                        h1 = (nlo + 1) // 2
                        segs += [(0, h1, 0), (h1, nlo, 0)] if nlo > 1 else [(0, nlo, 0)]
                    if nhi:
                        h2 = (nhi + 1) // 2
                        segs += (
                            [(nlo, nlo + h2, 1), (nlo + h2, cbg, 1)]
                            if nhi > 1
                            else [(nlo, cbg, 1)]
                        )
                    for s0, s1, hf in segs:
                        nc.gpsimd.dma_gather(
                            out_ap=gf[:, s0:s1, :],
                            in_ap=xb_full[0:HALF, :] if hf == 0 else xb_full[HALF:N, :],
                            idxs_ap=fidx_sb[:, (cb0 + s0) * 8 : (cb0 + s1) * 8],
                            num_idxs=(s1 - s0) * 128,
                            num_idxs_reg=(s1 - s0) * 128,
                            elem_size=128,
                            single_packet=False,
                            queue_num=qctr % NQ,
                        )
                        qctr += 1
                    oh = ohpool.tile([128, CBG_MAX, 128], F8, tag="oh")
                    nc.sync.dma_start(
                        oh.rearrange("p a b -> p (a b)")[:, 0 : cbg * 128],
                        oh_d[:, cb0 * 128 : (cb0 + cbg) * 128],
                    )
                    ohT = ohtpool.tile([128, CBG_MAX, 128], F8, tag="ohT")
                    nc.sync.dma_start(
                        ohT.rearrange("p a b -> p (a b)")[:, 0 : cbg * 128],
                        ohT_d[:, cb0 * 128 : (cb0 + cbg) * 128],
                    )

                    # per-edge a_dst via transposed one-hot x per-block vector
                    pa = ps_adst.tile([128, CBG_MAX], F32, space="PSUM")
                    for j in range(cbg):
                        nc.tensor.matmul(
                            pa[:, j : j + 1],
                            lhsT=ohT[:, j, :],
                            rhs=adst_sb[:, cb_block[cb0 + j] : cb_block[cb0 + j] + 1],
                            start=True,
                            stop=True,
                            skip_group_check=True,
                        )

                    # ee = exp(leakyrelu(|att_src|*t127 + a_dst'))
                    e1 = epool.tile([128, CBG_MAX], F32, tag="e1")
                    nc.vector.scalar_tensor_tensor(
                        e1[:, 0:cbg],
                        in0=gf[:, 0:cbg, 127],
                        scalar=float(na1),
                        in1=pa[:, 0:cbg],
                        op0=OP.mult,
                        op1=OP.add,
                    )
                    e3 = epool.tile([128, CBG_MAX], F32, tag="e3")
                    nc.vector.tensor_scalar(
                        e3[:, 0:cbg], e1[:, 0:cbg], NEG_SLOPE, None, OP.mult
                    )
                    nc.vector.tensor_tensor(
                        e3[:, 0:cbg], e3[:, 0:cbg], e1[:, 0:cbg], OP.max
                    )
                    ee = epool.tile([128, CBG_MAX], F32, tag="ee")
                    nc.scalar.activation(ee[:, 0:cbg], e3[:, 0:cbg], AF.Exp)

                    # rhs for the scatter matmuls: [t*ee (128) | ee | pad],
                    # built in one batched multiply + one strided column copy
                    gfe = gfepool.tile([128, CBG_MAX, 130], F16, tag="gfe")
                    nc.vector.tensor_tensor(
                        gfe[:, 0:cbg, 0:128],
                        gf[:, 0:cbg, :],
                        ee[:, 0:cbg].to_broadcast([128, cbg, 128]),
                        OP.mult,
                    )
                    nc.vector.tensor_copy(gfe[:, 0:cbg, 128], ee[:, 0:cbg])

                    # scatter matmuls per block
                    lo_off = 0
                    hi_off = nlo
                    for b in blocks:
                        rows = 128 if b < NBLK - 1 else LAST_ROWS
                        cbs = list(range(lo_off, lo_off + tlo[b])) + list(
                            range(hi_off, hi_off + thi[b])
                        )
                        lo_off += tlo[b]
                        hi_off += thi[b]
                        ps = ps_sc.tile([128, 129], F32, space="PSUM")
                        for j, cb in enumerate(cbs):
                            nc.tensor.matmul(
                                ps[:, :],
                                lhsT=oh[:, cb, :],
                                rhs=gfe[:, cb, 0:129],
                                start=(j == 0),
                                stop=(j == len(cbs) - 1),
                            )
                        recip = epool.tile([128, 1], F32, tag="recip")
                        nc.vector.reciprocal(recip[:], ps[:, 128:129])
                        scaled = blkpool.tile([128, D], F32, tag="scaled")
                        nc.scalar.activation(
                            scaled[:], ps[:, 0:D], AF.Copy, scale=recip[:, 0:1]
                        )
                        ptp = ps_tp.tile([128, 128], F32, space="PSUM")
                        nc.tensor.transpose(ptp[:], scaled[:], ident_sb[:])
                        scaledT = blkpool.tile([128, D], F16, tag="scaledT")
                        nc.scalar.copy(scaledT[:], ptp[:])
                        po = ps_out.tile([128, 128], F32, space="PSUM")
                        nc.tensor.matmul(
                            po[:], lhsT=scaledT[:], rhs=q_sb[:], start=True, stop=True
                        )
                        resid = blkpool.tile([128, D], F32, tag="resid")
                        nc.vector.tensor_tensor(
                            resid[:], po[:], x_tiles[b][:], OP.add
                        )
                        outt = blkpool.tile([128, D], F32, tag="outt")
                        nc.scalar.activation(outt[:], resid[:], AF.Relu)
                        nc.sync.dma_start(
                            out_shard[b * 128 : b * 128 + rows, :], outt[:rows, :]
                        )

    nc.compile()
    return nc


def _wrap_idx(idx):
    """int16 index list -> dma_gather SBUF layout [128, len/16]:
    index i lives at partitions {16g + i%16: g in 0..7}, column i//16."""
    L = len(idx)
    assert L % 16 == 0
    w = idx.reshape(L // 16, 16).T.astype(np.int16)      # [16, L/16]
    return np.tile(w, (8, 1))                            # [128, L/16]


def _host_prep(x, edge_index, ln_gamma, ln_beta, W, att_src, att_dst, bias):
    """Fold parameters, build rotation Q, bucket edges. Numpy only."""
    Wt = W.T.astype(np.float64)
    G = ln_gamma.astype(np.float64)[:, None] * Wt          # [D, D]
    crow = ln_beta.astype(np.float64) @ Wt                 # [D]
    a1 = att_src.astype(np.float64)
    a2 = att_dst.astype(np.float64)
    na1 = float(np.linalg.norm(a1))
    v_dst = G @ a2
    c_dst = float(crow @ a2)
    kc = float(bias.astype(np.float64) @ a1)

    # orthonormal Q with row 127 = att_src direction (row 126: att_dst comp,
    # kept only so Q is deterministic/well-conditioned)
    q127 = a1 / na1
    u = a2 - (a2 @ q127) * q127
    nu = np.linalg.norm(u)
    if nu > 1e-12:
        q126 = u / nu
        P = np.eye(D) - np.outer(q127, q127) - np.outer(q126, q126)
        Uq, _, _ = np.linalg.svd(P)
        Q = np.vstack([Uq[:, :126].T, q126[None, :], q127[None, :]])
    else:
        P = np.eye(D) - np.outer(q127, q127)
        Uq, _, _ = np.linalg.svd(P)
        Q = np.vstack([Uq[:, :127].T, q127[None, :]])

    c2feat = crow + bias.astype(np.float64)
    wextq = np.zeros((D, 129), np.float16)
    wextq[:, 0:128] = (G @ Q.T).astype(np.float16)
    wextq[:, 128] = v_dst.astype(np.float16)
    c2 = np.zeros((129,), np.float32)
    c2[0:128] = (c2feat @ Q.T).astype(np.float32)
    c2[128] = c_dst - kc
    c2b = np.broadcast_to(c2, (128, 129)).copy()

    ident = np.eye(128, dtype=np.float32)
    qmat = Q.astype(np.float16)

    # edges + self loops, sorted by (core, block, src-half)
    src = np.concatenate([edge_index[0], np.arange(N, dtype=np.int64)]).astype(np.int64)
    dst = np.concatenate([edge_index[1], np.arange(N, dtype=np.int64)]).astype(np.int64)
    core = dst // SHARD
    local = dst - core * SHARD
    blk = local // 128
    half = (src >= HALF).astype(np.int64)
    key = ((core * NBLK + blk) * 2 + half)
    order = np.argsort(key, kind="stable")
    src, dst, key = src[order], dst[order], key[order]
    counts = np.bincount(key, minlength=NCORES * NBLK * 2).reshape(NCORES, NBLK, 2)
    tiles = -(-counts // 128)                              # ceil
    tlo = tuple(int(t) for t in tiles[:, :, 0].max(axis=0))
    thi = tuple(int(t) for t in tiles[:, :, 1].max(axis=0))
    CB = sum(tlo) + sum(thi)

    feat_idx = np.zeros((NCORES, CB * 128), np.int16)
    oh = np.zeros((NCORES, 128, CB, 128), np.uint8)
    ohT = np.zeros((NCORES, 128, CB, 128), np.uint8)

    starts = np.zeros(NCORES * NBLK * 2 + 1, np.int64)
    starts[1:] = np.cumsum(counts.reshape(-1))

    # cb offset of each (block, half) segment, same for every core
    seg_off = {}
    cb0 = 0
    for g0 in range(0, NBLK, GBLK):
        blocks = list(range(g0, min(NBLK, g0 + GBLK)))
        off = cb0
        for b in blocks:
            seg_off[(b, 0)] = off
            off += tlo[b]
        for b in blocks:
            seg_off[(b, 1)] = off
            off += thi[b]
        cb0 = off
    assert cb0 == CB

    for c in range(NCORES):
        for b in range(NBLK):
            for hf in range(2):
                gi = (c * NBLK + b) * 2 + hf
                s, e = starts[gi], starts[gi + 1]
                n = int(e - s)
                if n == 0:
                    continue
                off = seg_off[(b, hf)]
                k = np.arange(n) + off * 128
                fi = (src[s:e] - hf * HALF).astype(np.int16)
                feat_idx[c, k] = fi
                p = k % 128
                t = k // 128
                r = (dst[s:e] - (c * SHARD + b * 128)).astype(np.int64)
                oh[c, p, t, r] = 1
                ohT[c, r, t, p] = 1

    oh8 = oh.astype(ml_dtypes.float8_e4m3fn).reshape(NCORES, 128, CB * 128)
    ohT8 = ohT.astype(ml_dtypes.float8_e4m3fn).reshape(NCORES, 128, CB * 128)

    in_maps = []
    for c in range(NCORES):
        xs = np.zeros((PAD_SHARD, D), np.float32)
        xs[0:SHARD] = x[c * SHARD : (c + 1) * SHARD]
        in_maps.append(
            {
                "x_shard": xs,
                "wextq": wextq,
                "c2q": c2b,
                "ident": ident,
                "qmat": qmat,
                "feat_idx": _wrap_idx(feat_idx[c]),
                "oh_d": np.ascontiguousarray(oh8[c]),
                "ohT_d": np.ascontiguousarray(ohT8[c]),
            }
        )
    return tlo, thi, na1, in_maps


_PROGRAM_CACHE = {}


def kernel(x, edge_index, edge_attr, h, batch, ln_gamma, ln_beta, W, att_src,
           att_dst, bias):
    x = np.asarray(x, dtype=np.float32)
    edge_index = np.asarray(edge_index)
    h = np.asarray(h)
    ln_gamma = np.asarray(ln_gamma, dtype=np.float32)
    ln_beta = np.asarray(ln_beta, dtype=np.float32)
    W = np.asarray(W, dtype=np.float32)
    att_src = np.asarray(att_src, dtype=np.float32)
    att_dst = np.asarray(att_dst, dtype=np.float32)
    bias = np.asarray(bias, dtype=np.float32)

    tlo, thi, na1, in_maps = _host_prep(
        x, edge_index, ln_gamma, ln_beta, W, att_src, att_dst, bias
    )
    key = (tlo, thi, round(na1, 6))
    if key not in _PROGRAM_CACHE:
        _PROGRAM_CACHE[key] = _build_program(tlo, thi, na1)
    nc = _PROGRAM_CACHE[key]

    res = run_bass_kernel_spmd(nc, in_maps, core_ids=list(range(NCORES)))
    out = np.concatenate([res.results[c]["out_shard"] for c in range(NCORES)], axis=0)
    return out, h


# revision 14
# speedup vs baseline: 2.5491x; 1.0181x over previous
"""GAT layer (LayerNorm -> GATConv(heads=1) -> residual ReLU) on 8 trn2 NeuronCores.

Sharding: destination-node parallel. Each core owns N/8 contiguous nodes,
computes the node transform for its shard, AllGathers the transformed table,
then processes the edges whose destination lands in its shard.

Key design points (v2, rebuilt after profiling the 768B-row baseline):
- The node table is fp16, 256 B/row (the dma_gather minimum): a host-side
  orthonormal rotation Q puts att_src along coordinate 127, so the gathered
  row IS [rotated feats | a_src] with zero extra columns; the rotation is
  undone after the softmax-weighted scatter by one 128x128 matmul per dst
  block (Q is orthogonal, applied to the accumulated sums).
- No per-edge a_dst gather: a_dst per edge = ohT_cb^T @ adst_block via a
  1-column matmul per 128-edge column block, with one-hot tables streamed
  from the host as fp8 (exact 0/1).
- No DVE one-hot builds: the scatter matmul uses lhsT = host fp8 one-hot,
  rhs = gathered rows * ee (folded on ACT/DVE), with a constant ones column
  in the rhs producing the softmax denominator in the same matmul.
- Feature gathers round-robin over 4 SWDGE queues to overlap DMA drains.
"""

import numpy as np
import ml_dtypes

import concourse.bacc as bacc
import concourse.mybir as mybir
import concourse.tile as tile
from concourse.bass_utils import run_bass_kernel_spmd

F32 = mybir.dt.float32
F16 = mybir.dt.float16
F8 = mybir.dt.float8e4
I16 = mybir.dt.int16
AX = mybir.AxisListType
OP = mybir.AluOpType
AF = mybir.ActivationFunctionType

N = 50000
D = 128
E = 600000
NCORES = 8
SHARD = N // NCORES            # 6250
NBLK = (SHARD + 127) // 128    # 49 dst blocks per core
PAD_SHARD = NBLK * 128         # 6272
LAST_ROWS = SHARD - (NBLK - 1) * 128  # 106
HALF = 32768                   # int16 index split point for the global table
NEG_SLOPE = 0.2
LN_EPS = 1e-5
GBLK = 4                       # dst blocks per gather group
NQ = 4                         # SWDGE queues for gathers


def _build_program(tlo, thi, na1):
    """One SPMD program; per-core behaviour differs only through its inputs."""
    nc = bacc.Bacc("TRN2", num_devices=NCORES, debug=False, num_swdge_queues=NQ)

    CB = sum(tlo) + sum(thi)   # total column-blocks (tiles) per core

    x_shard = nc.dram_tensor("x_shard", [PAD_SHARD, D], F32, kind="ExternalInput")
    wextq = nc.dram_tensor("wextq", [D, 129], F16, kind="ExternalInput")
    c2q = nc.dram_tensor("c2q", [128, 129], F32, kind="ExternalInput")
    ident = nc.dram_tensor("ident", [128, 128], F32, kind="ExternalInput")
    qmat = nc.dram_tensor("qmat", [128, 128], F16, kind="ExternalInput")
    feat_idx = nc.dram_tensor("feat_idx", [128, CB * 8], I16, kind="ExternalInput")
    oh_d = nc.dram_tensor("oh_d", [128, CB * 128], F8, kind="ExternalInput")
    ohT_d = nc.dram_tensor("ohT_d", [128, CB * 128], F8, kind="ExternalInput")
    out_shard = nc.dram_tensor("out_shard", [SHARD, D], F32, kind="ExternalOutput")

    # group structure (static, identical on every core)
    groups = []
    cb0 = 0
    for g0 in range(0, NBLK, GBLK):
        blocks = list(range(g0, min(NBLK, g0 + GBLK)))
        nlo = sum(tlo[b] for b in blocks)
        nhi = sum(thi[b] for b in blocks)
        groups.append((blocks, cb0, nlo, nhi))
        cb0 += nlo + nhi
    assert cb0 == CB
    CBG_MAX = max(nlo + nhi for _, _, nlo, nhi in groups)

    # cb -> owning block (within its group), same order the host uses
    cb_block = [0] * CB
    for blocks, cb0g, nlo, nhi in groups:
        off = cb0g
        for b in blocks:
            for _ in range(tlo[b]):
                cb_block[off] = b
                off += 1
        for b in blocks:
            for _ in range(thi[b]):
                cb_block[off] = b
                off += 1

    with tile.TileContext(nc) as tc:
        with (
            tc.tile_pool(name="dram", bufs=1, space="DRAM") as dram,
            tc.tile_pool(name="consts", bufs=1) as cpool,
            tc.tile_pool(name="xres", bufs=1) as xpool,
        ):
            xb_shard = dram.tile([SHARD, D], F16)
            xb_full = dram.tile([N, D], F16, addr_space="Shared")

            ident_sb = cpool.tile([128, 128], F32)
            nc.sync.dma_start(ident_sb[:], ident[:, :])
            q_sb = cpool.tile([128, 128], F16)
            nc.sync.dma_start(q_sb[:], qmat[:, :])
            wext_sb = cpool.tile([D, 129], F16)
            nc.sync.dma_start(wext_sb[:], wextq[:, :])
            identh_sb = cpool.tile([128, 128], F16)
            nc.scalar.copy(identh_sb[:], ident_sb[:])
            c2_sb = cpool.tile([128, 129], F32)
            nc.sync.dma_start(c2_sb[:], c2q[:, :])
            eps_sb = cpool.tile([128, 1], F32)
            nc.vector.memset(eps_sb[:], LN_EPS)
            fidx_sb = cpool.tile([128, CB * 8], I16)
            nc.sync.dma_start(fidx_sb[:], feat_idx[:, :])
            adst_sb = cpool.tile([128, NBLK], F16)

            x_tiles = []
            for i in range(NBLK):
                xt = xpool.tile([128, D], F32, tag=f"xres{i}")
                nc.sync.dma_start(xt[:], x_shard[i * 128 : (i + 1) * 128, :])
                x_tiles.append(xt)

            # ---------------- Phase A: node transform on own shard ---------
            with (
                tc.tile_pool(name="a_small", bufs=8) as spool,
                tc.tile_pool(name="a_sq", bufs=2) as sqpool,
                tc.tile_pool(name="a_xnp", bufs=3) as xnppool,
                tc.tile_pool(name="a_xnpT", bufs=3) as xnptpool,
                tc.tile_pool(name="a_xpe", bufs=3) as xpepool,
                tc.tile_pool(name="a_tb", bufs=3) as tbpool,
                tc.tile_pool(name="a_ps_t", bufs=2, space="PSUM") as psa,
                tc.tile_pool(name="a_ps_m", bufs=2, space="PSUM") as psb,
            ):
                for i in range(NBLK):
                    xt = x_tiles[i]
                    rows = 128 if i < NBLK - 1 else LAST_ROWS
                    sumx = spool.tile([128, 1], F32, tag="sumx")
                    nc.vector.tensor_reduce(sumx[:], xt[:], AX.X, OP.add)
                    sqj = sqpool.tile([128, D], F32)
                    ssq = spool.tile([128, 1], F32, tag="ssq")
                    nc.scalar.activation(sqj[:], xt[:], AF.Square, accum_out=ssq[:])
                    mu = spool.tile([128, 1], F32, tag="mu")
                    nc.vector.tensor_scalar(mu[:], sumx[:], 1.0 / D, None, OP.mult)
                    m2 = spool.tile([128, 1], F32, tag="m2")
                    nc.vector.tensor_tensor(m2[:], mu[:], mu[:], OP.mult)
                    var = spool.tile([128, 1], F32, tag="var")
                    nc.vector.tensor_scalar(
                        var[:], ssq[:], 1.0 / D, m2[:, 0:1], OP.mult, OP.subtract
                    )
                    std = spool.tile([128, 1], F32, tag="std")
                    nc.scalar.activation(std[:], var[:], AF.Sqrt, bias=eps_sb[:, 0:1])
                    rstd = spool.tile([128, 1], F32, tag="rstd")
                    nc.vector.reciprocal(rstd[:], std[:])
                    xnp = xnppool.tile([128, D], F16)
                    nc.vector.tensor_scalar(
                        xnp[:], xt[:], mu[:, 0:1], rstd[:, 0:1], OP.subtract, OP.mult
                    )
                    pt = psa.tile([128, 128], F16, space="PSUM")
                    nc.tensor.transpose(pt[:], xnp[:], identh_sb[:])
                    xnpT = xnptpool.tile([128, 128], F16)
                    nc.scalar.copy(xnpT[:], pt[:])
                    pm = psb.tile([128, 129], F32, space="PSUM")
                    nc.tensor.matmul(
                        pm[:], lhsT=xnpT[:], rhs=wext_sb[:], start=True, stop=True
                    )
                    xpe = xpepool.tile([128, 129], F32)
                    nc.vector.tensor_tensor(xpe[:], pm[:], c2_sb[:], OP.add)
                    tb = tbpool.tile([128, D], F16, tag="tb")
                    nc.scalar.copy(tb[:], xpe[:, 0:128])
                    nc.sync.dma_start(
                        xb_shard[i * 128 : i * 128 + rows, :], tb[:rows, :]
                    )
                    nc.scalar.copy(adst_sb[:, i : i + 1], xpe[:, 128:129])

            nc.gpsimd.collective_compute(
                "AllGather",
                OP.bypass,
                replica_groups=[list(range(NCORES))],
                ins=[xb_shard[:, :]],
                outs=[xb_full[:, :]],
            )

            # ---------------- Phase B: edge aggregation --------------------
            with (
                tc.tile_pool(name="b_g", bufs=3) as gpool,
                tc.tile_pool(name="b_oh", bufs=3) as ohpool,
                tc.tile_pool(name="b_ohT", bufs=3) as ohtpool,
                tc.tile_pool(name="b_e", bufs=2) as epool,
                tc.tile_pool(name="b_gfe", bufs=3) as gfepool,
                tc.tile_pool(name="b_blk", bufs=4) as blkpool,
                tc.tile_pool(name="b_ps_a", bufs=2, space="PSUM") as ps_adst,
                tc.tile_pool(name="b_ps_s", bufs=2, space="PSUM") as ps_sc,
                tc.tile_pool(name="b_ps_t", bufs=2, space="PSUM") as ps_tp,
                tc.tile_pool(name="b_ps_o", bufs=2, space="PSUM") as ps_out,
            ):
                qctr = 0
                for gi, (blocks, cb0, nlo, nhi) in enumerate(groups):
                    cbg = nlo + nhi
                    gf = gpool.tile([128, CBG_MAX, 128], F16, tag="gf")
                    # split each half-table gather in two on separate SWDGE
                    # queues so their DMA drains overlap
                    segs = []
                    if nlo:
                        h1 = (nlo + 1) // 2
                        segs += [(0, h1, 0), (h1, nlo, 0)] if nlo > 1 else [(0, nlo, 0)]
                    if nhi:
                        h2 = (nhi + 1) // 2
                        segs += (
                            [(nlo, nlo + h2, 1), (nlo + h2, cbg, 1)]
                            if nhi > 1
                            else [(nlo, cbg, 1)]
                        )
                    for s0, s1, hf in segs:
                        nc.gpsimd.dma_gather(
                            out_ap=gf[:, s0:s1, :],
                            in_ap=xb_full[0:HALF, :] if hf == 0 else xb_full[HALF:N, :],
                            idxs_ap=fidx_sb[:, (cb0 + s0) * 8 : (cb0 + s1) * 8],
                            num_idxs=(s1 - s0) * 128,
                            num_idxs_reg=(s1 - s0) * 128,
                            elem_size=128,
                            single_packet=False,
                            queue_num=qctr % NQ,
                        )
                        qctr += 1
                    oh = ohpool.tile([128, CBG_MAX, 128], F8, tag="oh")
                    nc.sync.dma_start(
                        oh.rearrange("p a b -> p (a b)")[:, 0 : cbg * 128],
                        oh_d[:, cb0 * 128 : (cb0 + cbg) * 128],
                    )
                    ohT = ohtpool.tile([128, CBG_MAX, 128], F8, tag="ohT")
                    nc.sync.dma_start(
                        ohT.rearrange("p a b -> p (a b)")[:, 0 : cbg * 128],
                        ohT_d[:, cb0 * 128 : (cb0 + cbg) * 128],
                    )

                    # per-edge a_dst via transposed one-hot x per-block vector
                    pa = ps_adst.tile([128, CBG_MAX], F32, space="PSUM")
                    for j in range(cbg):
                        nc.tensor.matmul(
                            pa[:, j : j + 1],
                            lhsT=ohT[:, j, :],
                            rhs=adst_sb[:, cb_block[cb0 + j] : cb_block[cb0 + j] + 1],
                            start=True,
                            stop=True,
                            skip_group_check=True,
                        )

                    # ee = exp(leakyrelu(|att_src|*t127 + a_dst'))
                    e1 = epool.tile([128, CBG_MAX], F32, tag="e1")
                    nc.vector.scalar_tensor_tensor(
                        e1[:, 0:cbg],
                        in0=gf[:, 0:cbg, 127],
                        scalar=float(na1),
                        in1=pa[:, 0:cbg],
                        op0=OP.mult,
                        op1=OP.add,
                    )
                    e3 = epool.tile([128, CBG_MAX], F32, tag="e3")
                    nc.vector.scalar_tensor_tensor(
                        e3[:, 0:cbg],
                        in0=e1[:, 0:cbg],
                        scalar=NEG_SLOPE,
                        in1=e1[:, 0:cbg],
                        op0=OP.mult,
                        op1=OP.max,
                    )
                    ee = epool.tile([128, CBG_MAX], F32, tag="ee")
                    nc.scalar.activation(ee[:, 0:cbg], e3[:, 0:cbg], AF.Exp)

                    # rhs for the scatter matmuls: [t*ee (128) | ee | pad],
                    # built in one batched multiply + one strided column copy
                    gfe = gfepool.tile([128, CBG_MAX, 130], F16, tag="gfe")
                    nc.vector.tensor_tensor(
                        gfe[:, 0:cbg, 0:128],
                        gf[:, 0:cbg, :],
                        ee[:, 0:cbg].to_broadcast([128, cbg, 128]),
                        OP.mult,
                    )
                    nc.vector.tensor_copy(gfe[:, 0:cbg, 128], ee[:, 0:cbg])

                    # scatter matmuls per block
                    lo_off = 0
                    hi_off = nlo
                    for b in blocks:
                        rows = 128 if b < NBLK - 1 else LAST_ROWS
                        cbs = list(range(lo_off, lo_off + tlo[b])) + list(
                            range(hi_off, hi_off + thi[b])
                        )
                        lo_off += tlo[b]
                        hi_off += thi[b]
                        ps = ps_sc.tile([128, 129], F32, space="PSUM")
                        for j, cb in enumerate(cbs):
                            nc.tensor.matmul(
                                ps[:, :],
                                lhsT=oh[:, cb, :],
                                rhs=gfe[:, cb, 0:129],
                                start=(j == 0),
                                stop=(j == len(cbs) - 1),
                            )
                        recip = epool.tile([128, 1], F32, tag="recip")
                        nc.vector.reciprocal(recip[:], ps[:, 128:129])
                        scaled = blkpool.tile([128, D], F32, tag="scaled")
                        nc.scalar.activation(
                            scaled[:], ps[:, 0:D], AF.Copy, scale=recip[:, 0:1]
                        )
                        ptp = ps_tp.tile([128, 128], F32, space="PSUM")
                        nc.tensor.transpose(ptp[:], scaled[:], ident_sb[:])
                        scaledT = blkpool.tile([128, D], F16, tag="scaledT")
                        nc.scalar.copy(scaledT[:], ptp[:])
                        po = ps_out.tile([128, 128], F32, space="PSUM")
                        nc.tensor.matmul(
                            po[:], lhsT=scaledT[:], rhs=q_sb[:], start=True, stop=True
                        )
                        resid = blkpool.tile([128, D], F32, tag="resid")
                        nc.vector.tensor_tensor(
                            resid[:], po[:], x_tiles[b][:], OP.add
                        )
                        outt = blkpool.tile([128, D], F32, tag="outt")
                        nc.scalar.activation(outt[:], resid[:], AF.Relu)
                        nc.sync.dma_start(
                            out_shard[b * 128 : b * 128 + rows, :], outt[:rows, :]
                        )

    nc.compile()
    return nc


def _wrap_idx(idx):
    """int16 index list -> dma_gather SBUF layout [128, len/16]:
    index i lives at partitions {16g + i%16: g in 0..7}, column i//16."""
    L = len(idx)
    assert L % 16 == 0
    w = idx.reshape(L // 16, 16).T.astype(np.int16)      # [16, L/16]
    return np.tile(w, (8, 1))                            # [128, L/16]


def _host_prep(x, edge_index, ln_gamma, ln_beta, W, att_src, att_dst, bias):
    """Fold parameters, build rotation Q, bucket edges. Numpy only."""
    Wt = W.T.astype(np.float64)
    G = ln_gamma.astype(np.float64)[:, None] * Wt          # [D, D]
    crow = ln_beta.astype(np.float64) @ Wt                 # [D]
    a1 = att_src.astype(np.float64)
    a2 = att_dst.astype(np.float64)
    na1 = float(np.linalg.norm(a1))
    v_dst = G @ a2
    c_dst = float(crow @ a2)
    kc = float(bias.astype(np.float64) @ a1)

    # orthonormal Q with row 127 = att_src direction (row 126: att_dst comp,
    # kept only so Q is deterministic/well-conditioned)
    q127 = a1 / na1
    u = a2 - (a2 @ q127) * q127
    nu = np.linalg.norm(u)
    if nu > 1e-12:
        q126 = u / nu
        P = np.eye(D) - np.outer(q127, q127) - np.outer(q126, q126)
        Uq, _, _ = np.linalg.svd(P)
        Q = np.vstack([Uq[:, :126].T, q126[None, :], q127[None, :]])
    else:
        P = np.eye(D) - np.outer(q127, q127)
        Uq, _, _ = np.linalg.svd(P)
        Q = np.vstack([Uq[:, :127].T, q127[None, :]])

    c2feat = crow + bias.astype(np.float64)
    wextq = np.zeros((D, 129), np.float16)
    wextq[:, 0:128] = (G @ Q.T).astype(np.float16)
    wextq[:, 128] = v_dst.astype(np.float16)
    c2 = np.zeros((129,), np.float32)
    c2[0:128] = (c2feat @ Q.T).astype(np.float32)
    c2[128] = c_dst - kc
    c2b = np.broadcast_to(c2, (128, 129)).copy()

    ident = np.eye(128, dtype=np.float32)
    qmat = Q.astype(np.float16)

    # edges + self loops, sorted by (core, block, src-half)
    src = np.concatenate([edge_index[0], np.arange(N, dtype=np.int64)]).astype(np.int64)
    dst = np.concatenate([edge_index[1], np.arange(N, dtype=np.int64)]).astype(np.int64)
    core = dst // SHARD
    local = dst - core * SHARD
    blk = local // 128
    half = (src >= HALF).astype(np.int64)
    key = ((core * NBLK + blk) * 2 + half)
    order = np.argsort(key, kind="stable")
    src, dst, key = src[order], dst[order], key[order]
    counts = np.bincount(key, minlength=NCORES * NBLK * 2).reshape(NCORES, NBLK, 2)
    tiles = -(-counts // 128)                              # ceil
    tlo = tuple(int(t) for t in tiles[:, :, 0].max(axis=0))
    thi = tuple(int(t) for t in tiles[:, :, 1].max(axis=0))
    CB = sum(tlo) + sum(thi)

    feat_idx = np.zeros((NCORES, CB * 128), np.int16)
    oh = np.zeros((NCORES, 128, CB, 128), np.uint8)
    ohT = np.zeros((NCORES, 128, CB, 128), np.uint8)

    starts = np.zeros(NCORES * NBLK * 2 + 1, np.int64)
    starts[1:] = np.cumsum(counts.reshape(-1))

    # cb offset of each (block, half) segment, same for every core
    seg_off = {}
    cb0 = 0
    for g0 in range(0, NBLK, GBLK):
        blocks = list(range(g0, min(NBLK, g0 + GBLK)))
        off = cb0
        for b in blocks:
            seg_off[(b, 0)] = off
            off += tlo[b]
        for b in blocks:
            seg_off[(b, 1)] = off
            off += thi[b]
        cb0 = off
    assert cb0 == CB

    for c in range(NCORES):
        for b in range(NBLK):
            for hf in range(2):
                gi = (c * NBLK + b) * 2 + hf
                s, e = starts[gi], starts[gi + 1]
                n = int(e - s)
                if n == 0:
                    continue
                off = seg_off[(b, hf)]
                k = np.arange(n) + off * 128
                fi = (src[s:e] - hf * HALF).astype(np.int16)
                feat_idx[c, k] = fi
                p = k % 128
                t = k // 128
                r = (dst[s:e] - (c * SHARD + b * 128)).astype(np.int64)
                oh[c, p, t, r] = 1
                ohT[c, r, t, p] = 1

    oh8 = oh.astype(ml_dtypes.float8_e4m3fn).reshape(NCORES, 128, CB * 128)
    ohT8 = ohT.astype(ml_dtypes.float8_e4m3fn).reshape(NCORES, 128, CB * 128)

    in_maps = []
    for c in range(NCORES):
        xs = np.zeros((PAD_SHARD, D), np.float32)
        xs[0:SHARD] = x[c * SHARD : (c + 1) * SHARD]
        in_maps.append(
            {
                "x_shard": xs,
                "wextq": wextq,
                "c2q": c2b,
                "ident": ident,
                "qmat": qmat,
                "feat_idx": _wrap_idx(feat_idx[c]),
                "oh_d": np.ascontiguousarray(oh8[c]),
                "ohT_d": np.ascontiguousarray(ohT8[c]),
            }
        )
    return tlo, thi, na1, in_maps


_PROGRAM_CACHE = {}


def kernel(x, edge_index, edge_attr, h, batch, ln_gamma, ln_beta, W, att_src,
           att_dst, bias):
    x = np.asarray(x, dtype=np.float32)
    edge_index = np.asarray(edge_index)
    h = np.asarray(h)
    ln_gamma = np.asarray(ln_gamma, dtype=np.float32)
    ln_beta = np.asarray(ln_beta, dtype=np.float32)
    W = np.asarray(W, dtype=np.float32)
    att_src = np.asarray(att_src, dtype=np.float32)
    att_dst = np.asarray(att_dst, dtype=np.float32)
    bias = np.asarray(bias, dtype=np.float32)

    tlo, thi, na1, in_maps = _host_prep(
        x, edge_index, ln_gamma, ln_beta, W, att_src, att_dst, bias
    )
    key = (tlo, thi, round(na1, 6))
    if key not in _PROGRAM_CACHE:
        _PROGRAM_CACHE[key] = _build_program(tlo, thi, na1)
    nc = _PROGRAM_CACHE[key]

    res = run_bass_kernel_spmd(nc, in_maps, core_ids=list(range(NCORES)))
    out = np.concatenate([res.results[c]["out_shard"] for c in range(NCORES)], axis=0)
    return out, h
